# revision 54
# baseline (speedup 1.0000x reference)
"""Trainium2 Bass kernel for nn_BDH_6313601925221 (sparse_attention).

Model (reference.py):
  x = LN(embed[idx])                                   (B=1, T=1024, D=256)
  repeat 6 layers (shared weights):
    x_sparse = relu(einsum('btd,hdn->bhtn', x, encoder))   N=8192, NH=4
    QR       = rope(x_sparse)                              interleaved-pair rotation
    scores   = einsum('bhtn,bhsn->bhts', QR, QR) * strict_causal
    yKV      = LN(einsum('bhts,bsd->bhtd', scores, x))
    y_sparse = relu(einsum('bhtd,hdn->bhtn', yKV, encoder_v))
    yMLP     = (x_sparse*y_sparse).transpose -> (T, NH*N) @ decoder
    x        = LN(x + LN(yMLP))
  logits = x @ lm_head

Distribution (8 cores): core c = (head h=c//2, latent-half eta=c%2).
Each core computes encoder/rope/scores over its 4096 latent dims.  The
score strips are NEVER exchanged: scores only feed yKV = scores @ x,
which is linear in scores, so each core computes a partial ykvT from its
local strips and a single pairwise AllReduce of ykvT [256,1024] (split
into two t-half chunks) replaces the baseline's four score-strip
AllReduces + DRAM round trip.

The inner LN on yKV is dropped (scale-invariance: relu is positively
homogeneous and the whole path to yMLP is linear in the per-token scale;
x rows are zero-mean so the mean term vanishes).  The deferred 1/std is
applied to the ym partials pre-AllReduce (exact, incl. eps).

Layer schedule (token-half pipelined):
  PHASE A:  th0 encoder+rope j-loop, pipelined A'-scores (kb<4, q<512)
            -> spill strips -> ykv half0 -> pair-AR0        [needs x_d16h th0]
  PHASE B:  th1 encoder+rope j-loop, pipelined B'-scores (kb<4, q>=512)
  PHASE C:  kb>=4 scores sweep -> ykv half1 -> pair-AR1
  pass0 (t 0:512)  -> ym AR ci0;  pass1 (512:768) -> ci1
  TAIL0 (d-major)  -> x_d16h th0 ready
  pass2 (768:1024) -> ci2;  TAIL1
  [NEXT LAYER PHASE A emitted here -- overlaps ci2 AR + TAIL2]
  TAIL2  -> x_d16h th1 complete -> [NEXT LAYER PHASE B ...]

The tail runs fully in d-major layout: column stats via PE-ones matmuls
+ gpsimd partition_broadcast; x_t16 (needed ~50us later by ykv) is
produced by [128,128] transpose DMAs off the critical path.

PSUM (8 banks): S1,S2,S3 = three [128,1024] f32 accumulators (2 banks
each) cycling through score strips / ykv halves / ym accumulation;
tagP = [128,512] bufs=2 (2 banks) for transient matmul outputs.
"""

import math
import sys

import numpy as np

for _p in ("/opt/trn_rl_repo",):
    if _p not in sys.path:
        sys.path.insert(0, _p)

import concourse.bass as bass
import concourse.mybir as mybir
import concourse.tile as tile
from concourse import bacc
from concourse import bass_utils

# ---------------------------------------------------------------- constants
D = 256
NH = 4
N = 8192
T = 1024
N_LAYER = 6
VOCAB = 256
THETA = 2 ** 16
EPS = 1e-5
NCORES = 8

NHALF = N // 2          # 4096 latent dims per core
NPAIR = NHALF // 2      # 2048 rope pairs per core
NT = NHALF // 128       # 32 local n-tiles of 128
NJ = NT // 2            # 16 pair-blocks (tile 2j = evens, 2j+1 = odds)
TB = T // 128           # 8 token blocks
DC = D // 128           # 2 d-chunks
TH = T // 2             # 512 token half

F16 = mybir.dt.float16
F32 = mybir.dt.float32
F8 = mybir.dt.float8e4
I32 = mybir.dt.int32
DR = mybir.MatmulPerfMode.DoubleRow
AX = mybir.AxisListType
ALU = mybir.AluOpType
ACTF = mybir.ActivationFunctionType

ALPHA = 1.0 / 512.0     # yKV pre-scale (overflow headroom; cancels exactly)
BETA = 1.0 / 16.0       # extra scale inside Square so sq fits fp16

# phase-3 / yMLP-AllReduce / tail chunks: (t-col lo, width, token blocks)
YM_CHUNKS = [(0, 512, (0, 1, 2, 3)), (512, 256, (4, 5)), (768, 256, (6, 7))]

# PSUM: S1 [128,1024], S2 [128,512], S3 [128,1024], tagP [128,512]x3
# A' strip PSUM layout: kb -> (S-tag, col offset); strip width (4-kb)*128
A_LAY = {0: ("S1", 0), 1: ("S1", 512), 2: ("S2", 0), 3: ("S2", 256)}
# B-loop strips (kb 0..4, q in [512,1024), width 512 each; kb4 rides along
# in the j-loop to keep it PE-bound)
B_LAY = {0: ("S1", 0), 1: ("S1", 512), 2: ("S3", 0), 3: ("S3", 512),
         4: ("S2", 0)}
# C strips (kb 5..7, width (8-kb)*128)
C_LAY = {5: ("S3", 0), 6: ("S3", 512), 7: ("S3", 896)}


def build_program(n_layer=N_LAYER):
    nc = bacc.Bacc("TRN2", target_bir_lowering=False, debug=False,
                   num_devices=NCORES)

    # ------------------------------------------------------------- I/O decl
    # x0 = LN(embed)[idx] precomputed on host (input prep, like the rope
    # tables); provided in both t-major and d-major layouts.
    x0t_i = nc.dram_tensor("x0_t", [T, D], F16, kind="ExternalInput")
    x0d_i = nc.dram_tensor("x0_d", [D, T], F16, kind="ExternalInput")
    enc_i = nc.dram_tensor("enc_sh", [D, NHALF], F16, kind="ExternalInput")
    encv_i = nc.dram_tensor("encv_sh", [D, NHALF], F16, kind="ExternalInput")
    dec_i = nc.dram_tensor("dec_sh", [NHALF, D], F16, kind="ExternalInput")
    lmh_i = nc.dram_tensor("lmh", [D, VOCAB], F16, kind="ExternalInput")
    # per pair-block row: [c_th0 | s_th0 | c_th1 | s_th1], each TH wide
    cos2_i = nc.dram_tensor("cos2_sh", [NPAIR, 2 * T], F16,
                            kind="ExternalInput")
    cmask_i = nc.dram_tensor("cmask", [128, 128], F16, kind="ExternalInput")
    out_o = nc.dram_tensor("logits", [T, VOCAB], F32, kind="ExternalOutput")

    pair_groups = [[2 * h, 2 * h + 1] for h in range(NH)]
    all_group = [list(range(NCORES))]

    with tile.TileContext(nc) as tc:
      with (
        tc.tile_pool(name="persist", bufs=1) as pp,
        tc.tile_pool(name="work", bufs=2) as wp,
        tc.tile_pool(name="psW", bufs=2, space="PSUM") as psW,
        tc.tile_pool(name="psAcc", bufs=1, space="PSUM") as psAcc,
        tc.tile_pool(name="dram", bufs=1, space="DRAM") as dp,
      ):
        # ------------------------------------------------- persistent SBUF
        enc_sb = [pp.tile([128, NHALF], F16, name=f"enc{d}", tag=f"enc{d}")
                  for d in range(DC)]
        encv_sb = [pp.tile([128, NHALF], F16, name=f"encv{d}", tag=f"encv{d}")
                   for d in range(DC)]
        dec_sb = [pp.tile([128, D], F16, name=f"dec{i}", tag=f"dec{i}")
                  for i in range(NT)]
        # QR stored fp8 (e4m3) in DoubleRow layout: [128, k-subtile, t];
        # subtile 0 = even-parity latent tile (qe), 1 = odd (qo).  The
        # scores matmul runs in fp8 DoubleRow at 0.5 cycles/row -- final
        # error impact measured at ~1.2e-3 (errors average through yKV).
        QR8 = [pp.tile([128, 2, T], F8, name=f"qr8_{p}", tag=f"qr8_{p}")
               for p in range(NJ)]
        # local score strips in SBUF (fp16, diag-masked)
        ST_lo = [pp.tile([128, (4 - kb) * 128], F16, name=f"stl{kb}",
                         tag=f"stl{kb}") for kb in range(4)]
        ST_hi = [pp.tile([128, min(512, (8 - kb) * 128)], F16,
                         name=f"sth{kb}", tag=f"sth{kb}") for kb in range(8)]
        x_t16 = [pp.tile([128, D], F16, name=f"xt16_{i}", tag=f"xt16_{i}")
                 for i in range(TB)]
        x_d16h = [[pp.tile([128, TH], F16, name=f"xd16_{th}_{i}",
                           tag=f"xd16_{th}_{i}") for i in range(DC)]
                  for th in range(2)]
        ykvT = [pp.tile([128, T], F16, name=f"ykvT{i}", tag=f"ykvT{i}")
                for i in range(DC)]
        cmask = pp.tile([128, 128], F16, name="cmaskt", tag="cmaskt")
        eps_t = pp.tile([128, 1], F32, name="eps_t", tag="eps_t")
        ones_t = pp.tile([128, 1], F16, name="ones_t", tag="ones_t")
        eps2_t = pp.tile([1, 1], F32, name="eps2_t", tag="eps2_t")
        lmh_sb = [pp.tile([128, VOCAB], F16, name=f"lmh{d}", tag=f"lmh{d}")
                  for d in range(DC)]

        # ---------------------------------------------------- DRAM buffers
        xs_spill = [dp.tile([NHALF, T], F16, name=f"xs_spill{i}")
                    for i in range(2)]
        ykv_ins = [dp.tile([D, TH], F16, name=f"ykv_in{h}", tag=f"ykv_in{h}")
                   for h in range(2)]
        ykv_outs = [[dp.tile([D, TH], F16, name=f"ykv_out{l}_{h}",
                             tag=f"ykv_out{l}_{h}")
                     for h in range(2)] for l in range(n_layer)]
        ym_ins = [dp.tile([D, w], F16, name=f"ym_in{ci}", tag=f"ym_in{ci}")
                  for ci, (_, w, _) in enumerate(YM_CHUNKS)]
        ym_outs = [[dp.tile([D, w], F16, name=f"ym_out{l}_{ci}",
                            tag=f"ym_out{l}_{ci}", addr_space="Shared")
                    for ci, (_, w, _) in enumerate(YM_CHUNKS)]
                   for l in range(n_layer)]

        def psw(name, shape=(128, 512), dtype=F32):
            return psW.tile(list(shape), dtype, name=name, tag="ps_w",
                            padded_shape=[128, 512], bufs=3)

        def sacc(tag, name):
            w = 512 if tag == "S2" else 1024
            return psAcc.tile([128, w], F32, name=name, tag=tag)

        # ------------------------------------------------------ load consts
        nc.gpsimd.memset(eps_t[:], EPS)
        nc.gpsimd.memset(ones_t[:], 1.0)
        nc.gpsimd.memset(eps2_t[:], EPS * ALPHA * ALPHA)
        nc.sync.dma_start(cmask[:], cmask_i[:, :])
        for i in range(NT):
            nc.scalar.dma_start(dec_sb[i][:], dec_i[128 * i:128 * (i + 1), :])

        # ------------------------------------------------------- x0 loads
        for d in range(DC):
            nc.sync.dma_start(enc_sb[d][:], enc_i[128 * d:128 * (d + 1), :])
            nc.sync.dma_start(encv_sb[d][:],
                              encv_i[128 * d:128 * (d + 1), :])
            nc.sync.dma_start(lmh_sb[d][:], lmh_i[128 * d:128 * (d + 1), :])
        for tb in range(TB):
            nc.sync.dma_start(x_t16[tb][:], x0t_i[128 * tb:128 * (tb + 1), :])
        for th in range(2):
            for d in range(DC):
                nc.sync.dma_start(x_d16h[th][d][:],
                                  x0d_i[128 * d:128 * (d + 1),
                                        TH * th:TH * (th + 1)])

        # ===================================================== layer pieces
        def emit_phase_th(layer, th, acc, j_lo=0, j_hi=NJ):
            """Encoder+rope j-loop segment [j_lo, j_hi) for token half `th`,
            with depth-2 pipelined scores into `acc` (A' strips for th=0,
            B' strips for th=1).  Flushes trailing pairs when j_hi==NJ."""
            xs = xs_spill[layer % 2]
            lay = A_LAY if th == 0 else B_LAY

            def scores_pair(p):
                for kb in range(4 if th == 0 else 5):
                    tag, off = lay[kb]
                    if th == 0:
                        w = (4 - kb) * 128
                        q0 = 128 * kb
                    else:
                        w = 512
                        q0 = 512
                    nc.tensor.matmul(
                        acc[tag][:, off:off + w],
                        QR8[p][:, :, 128 * kb:128 * (kb + 1)],
                        QR8[p][:, :, q0:q0 + w],
                        start=(p == 0), stop=(p == NJ - 1),
                        perf_mode=DR)

            for j in range(j_lo, j_hi):
                cs2 = wp.tile([128, T], F16, name="cs2", tag="cs2", bufs=4)
                nc.sync.dma_start(cs2[:],
                                  cos2_i[128 * j:128 * (j + 1),
                                         T * th:T * (th + 1)])
                xs2 = wp.tile([128, T], F16, name="xs2", tag="xs2", bufs=5)
                for par in range(2):
                    nt = 2 * j + par
                    ps_e = psw(f"ps_enc_{layer}_{th}_{nt}")
                    for d in range(DC):
                        nc.tensor.matmul(
                            ps_e[:],
                            enc_sb[d][:, 128 * nt:128 * (nt + 1)],
                            x_d16h[th][d][:],
                            start=(d == 0), stop=(d == DC - 1))
                    nc.scalar.activation(xs2[:, TH * par:TH * (par + 1)],
                                         ps_e[:], ACTF.Relu)
                nc.sync.dma_start(
                    xs[256 * j:256 * (j + 1),
                       TH * th:TH * (th + 1)].rearrange(
                        "(b p) n -> p b n", p=128),
                    xs2[:].rearrange("p (b n) -> p b n", n=TH))
                # rope: cs2 = [c|s], xs2 = [xe|xo]; m2 split DVE/GpSimd to
                # balance engine load (j-loop is rope-throughput paced).
                # qe/qo land in fp16 (1-byte writes are slow on DVE/GpSimd);
                # one wide ACT copy converts both into the fp8 DR layout.
                m1 = wp.tile([128, T], F16, name="m1", tag="rope_m", bufs=6)
                nc.vector.tensor_mul(m1[:], xs2[:], cs2[:])
                m3 = wp.tile([128, T], F16, name="m3", tag="rope_q", bufs=6)
                nc.vector.tensor_sub(m3[:, 0:TH], m1[:, 0:TH], m1[:, TH:T])
                m2 = wp.tile([128, T], F16, name="m2", tag="rope_m", bufs=6)
                nc.gpsimd.tensor_mul(m2[:, 0:TH], xs2[:, TH:T], cs2[:, 0:TH])
                nc.vector.tensor_mul(m2[:, TH:T], xs2[:, 0:TH], cs2[:, TH:T])
                nc.gpsimd.tensor_add(m3[:, TH:T], m2[:, 0:TH], m2[:, TH:T])
                nc.scalar.activation(
                    QR8[j][:, :, TH * th:TH * (th + 1)],
                    m3[:].rearrange("p (s n) -> p s n", s=2),
                    ACTF.Copy)
                if j >= 4:
                    scores_pair(j - 4)
            if j_hi == NJ:
                for p in (NJ - 4, NJ - 3, NJ - 2, NJ - 1):
                    scores_pair(p)

        def spill_A(acc):
            for kb in range(4):
                tag, off = A_LAY[kb]
                w = (4 - kb) * 128
                nc.vector.tensor_copy(ST_lo[kb][:], acc[tag][:, off:off + w])
                nc.vector.tensor_mul(ST_lo[kb][:, 0:128],
                                     ST_lo[kb][:, 0:128], cmask[:])

        def spill_B(acc):
            for kb in range(5):
                tag, off = B_LAY[kb]
                nc.vector.tensor_copy(ST_hi[kb][:], acc[tag][:, off:off + 512])
            nc.vector.tensor_mul(ST_hi[4][:, 0:128],
                                 ST_hi[4][:, 0:128], cmask[:])

        def spill_C(acc):
            for kb in range(5, 8):
                tag, off = C_LAY[kb]
                w = (8 - kb) * 128
                nc.vector.tensor_copy(ST_hi[kb][:], acc[tag][:, off:off + w])
                nc.vector.tensor_mul(ST_hi[kb][:, 0:128],
                                     ST_hi[kb][:, 0:128], cmask[:])

        def scores_C(acc):
            for p in range(NJ):
                for kb in range(5, 8):
                    tag, off = C_LAY[kb]
                    w = (8 - kb) * 128
                    nc.tensor.matmul(
                        acc[tag][:, off:off + w],
                        QR8[p][:, :, 128 * kb:128 * (kb + 1)],
                        QR8[p][:, :, 128 * kb:128 * kb + w],
                        start=(p == 0), stop=(p == NJ - 1),
                        perf_mode=DR)

        def ykv_half0(layer, yh):
            # ykvT partial, cols q in [0,512): strips kb 0..3 (ST_lo)
            for dc in range(DC):
                for kb in range(4):
                    w = (4 - kb) * 128
                    nc.tensor.matmul(
                        yh[:, 512 * dc + 128 * kb:512 * dc + 512],
                        x_t16[kb][:, 128 * dc:128 * (dc + 1)],
                        ST_lo[kb][:, 0:w],
                        start=(kb == 0), stop=(kb == 3))
            for dc in range(DC):
                yk = wp.tile([128, TH], F16, name="yk0", tag="yk_sb", bufs=2)
                nc.scalar.mul(yk[:], yh[:, 512 * dc:512 * dc + 512], ALPHA)
                nc.scalar.dma_start(ykv_ins[0][128 * dc:128 * (dc + 1), :],
                                    yk[:])
            nc.gpsimd.collective_compute(
                "AllReduce", ALU.add, replica_groups=pair_groups,
                ins=[ykv_ins[0].opt()], outs=[ykv_outs[layer][0].opt()])

        def ykv_half1(layer, yh):
            # ykvT partial, cols q in [512,1024): strips kb 0..7 (ST_hi)
            for dc in range(DC):
                for kb in range(8):
                    a = max(512, 128 * kb)
                    w = 1024 - a
                    nc.tensor.matmul(
                        yh[:, 512 * dc + a - 512:512 * dc + 512],
                        x_t16[kb][:, 128 * dc:128 * (dc + 1)],
                        ST_hi[kb][:, 0:w],
                        start=(kb == 0), stop=(kb == 7))
            for dc in range(DC):
                yk = wp.tile([128, TH], F16, name="yk1", tag="yk_sb", bufs=2)
                nc.scalar.mul(yk[:], yh[:, 512 * dc:512 * dc + 512], ALPHA)
                nc.scalar.dma_start(ykv_ins[1][128 * dc:128 * (dc + 1), :],
                                    yk[:])
            nc.gpsimd.collective_compute(
                "AllReduce", ALU.add, replica_groups=pair_groups,
                ins=[ykv_ins[1].opt()], outs=[ykv_outs[layer][1].opt()])

        def load_stats_half(layer, h):
            # load reduced ykvT half and compute per-token 1/std (deferred
            # inner LayerNorm; rows zero-mean so var = E[y^2])
            for dc in range(DC):
                nc.sync.dma_start(
                    ykvT[dc][:, TH * h:TH * (h + 1)],
                    ykv_outs[layer][h][128 * dc:128 * (dc + 1), :])
            ssq_ps = psW.tile([1, 512], F32, name=f"ssq_{layer}_{h}",
                              tag="ps_w", padded_shape=[128, 512], bufs=3)
            for dc in range(DC):
                sqt = wp.tile([128, TH], F16, name="sqt", tag="sqt", bufs=1)
                nc.scalar.activation(sqt[:], ykvT[dc][:, TH * h:TH * (h + 1)],
                                     ACTF.Square, scale=BETA)
                nc.tensor.matmul(ssq_ps[:], ones_t[:], sqt[:],
                                 start=(dc == 0), stop=(dc == DC - 1))
            std_row = wp.tile([1, TH], F32, name="std_row", tag="r_a",
                              bufs=2)
            nc.scalar.activation(std_row[:], ssq_ps[:], ACTF.Sqrt,
                                 bias=eps2_t[:],
                                 scale=1.0 / (D * BETA * BETA))
            inv_row = wp.tile([1, TH], F32, name="inv_row", tag="r_b",
                              bufs=2)
            nc.vector.reciprocal_approx_fast(inv_row[:], std_row[:])
            ib = wp.tile([128, TH], F32, name=f"inv_b{h}", tag=f"inv_b{h}",
                         bufs=1)
            nc.gpsimd.partition_broadcast(ib[:], inv_row[:])
            return ib

        def phase3_pass(layer, ci, ym_acc):
            lo, w, _ = YM_CHUNKS[ci]
            xs = xs_spill[layer % 2]
            prev = None
            xs_c2 = None

            def emit_ym(nt, xy):
                for dh in range(DC):
                    nc.tensor.matmul(
                        ym_acc[dh][:, lo:lo + w],
                        dec_sb[nt][:, 128 * dh:128 * (dh + 1)],
                        xy[:], start=(nt == 0), stop=(nt == NT - 1))

            for nt in range(NT):
                ps_v = psw(f"ps_ysp_{layer}_{nt}_{ci}", (128, w))
                for d in range(DC):
                    nc.tensor.matmul(
                        ps_v[:], encv_sb[d][:, 128 * nt:128 * (nt + 1)],
                        ykvT[d][:, lo:lo + w],
                        start=(d == 0), stop=(d == DC - 1))
                if nt % 2 == 0:
                    xs_c2 = wp.tile([128, 2 * w], F16, name="xs_c",
                                    tag="xs_c", bufs=3,
                                    padded_shape=[128, 1024])
                    nc.sync.dma_start(
                        xs_c2[:].rearrange("p (b n) -> p b n", n=w),
                        xs[128 * nt:128 * (nt + 2), lo:lo + w].rearrange(
                            "(b p) n -> p b n", p=128))
                xs_ap = xs_c2[:, w * (nt % 2):w * (nt % 2) + w]
                xy = wp.tile([128, w], F16, name="xy", tag="xy", bufs=4,
                             padded_shape=[128, 512])
                if ci == 0 and nt % 2 == 1:
                    # offload every other xy: relu on ACT (PSUM-capable),
                    # multiply on GpSimd -- DVE is otherwise the pacer
                    ys = wp.tile([128, w], F16, name="ys", tag="ys", bufs=2,
                                 padded_shape=[128, 512])
                    nc.scalar.activation(ys[:], ps_v[:], ACTF.Relu)
                    nc.gpsimd.tensor_mul(xy[:], ys[:], xs_ap)
                else:
                    nc.vector.scalar_tensor_tensor(
                        xy[:], ps_v[:], 0.0, xs_ap, op0=ALU.max, op1=ALU.mult)
                if prev is not None:
                    emit_ym(*prev)
                prev = (nt, xy)
            emit_ym(*prev)

        def ym_reduce(layer, ci, ib, ym_acc):
            lo, w, _ = YM_CHUNKS[ci]
            for dh in range(DC):
                ym_sb = wp.tile([128, w], F16, name=f"ym_sb{ci}",
                                tag="ym_sb", bufs=2, padded_shape=[128, 512])
                nc.vector.tensor_mul(ym_sb[:], ym_acc[dh][:, lo:lo + w],
                                     ib[:, lo % 512:lo % 512 + w])
                nc.scalar.dma_start(ym_ins[ci][128 * dh:128 * (dh + 1), :],
                                    ym_sb[:])
            nc.gpsimd.collective_compute(
                "AllReduce", ALU.add, replica_groups=all_group,
                ins=[ym_ins[ci].opt()], outs=[ym_outs[layer][ci].opt()])

        def tail_first(layer, ci):
            """d-major tail, part 1: load u, column stats (PE-ones), first
            LN chain up to v = (u - mu)*inv1 + x_old.  No PE work after the
            mu/sq matmuls, so subsequent PE instructions (pass2 etc.) are
            not blocked behind the DVE latency chain."""
            lo, w, tbs = YM_CHUNKS[ci]
            th = lo // TH
            l0 = lo - TH * th
            ym_out = ym_outs[layer][ci]
            u = [wp.tile([128, w], F16, name=f"u{ci}_{dc}", tag=f"u_{dc}",
                         bufs=1, padded_shape=[128, 512]) for dc in range(DC)]
            for dc in range(DC):
                nc.sync.dma_start(u[dc][:],
                                  ym_out[128 * dc:128 * (dc + 1), :])
            # column stats over d (partition dim) via PE-ones matmuls
            mu_ps = psw(f"mu_ps_{layer}_{ci}", (1, w))
            for dc in range(DC):
                nc.tensor.matmul(mu_ps[:], ones_t[:], u[dc][:],
                                 start=(dc == 0), stop=(dc == DC - 1))
            sq_ps = psw(f"sq_ps_{layer}_{ci}", (1, w))
            for dc in range(DC):
                squ = wp.tile([128, w], F16, name="squ", tag="squ", bufs=2,
                              padded_shape=[128, 512])
                nc.vector.tensor_mul(squ[:], u[dc][:], u[dc][:])
                nc.tensor.matmul(sq_ps[:], ones_t[:], squ[:],
                                 start=(dc == 0), stop=(dc == DC - 1))
            mu_row = wp.tile([1, w], F32, name="mu_row", tag="r_mu", bufs=2)
            nc.scalar.mul(mu_row[:], mu_ps[:], 1.0 / D)
            mu_b = wp.tile([128, w], F32, name="mu_b", tag="mu_b", bufs=1,
                           padded_shape=[128, 512])
            nc.gpsimd.partition_broadcast(mu_b[:], mu_row[:])
            msq = wp.tile([1, w], F32, name="msq", tag="r_a", bufs=2)
            nc.vector.tensor_mul(msq[:], mu_row[:], mu_row[:])
            # vr2 = ssq/D - mu^2  (one fused DVE op, PSUM source)
            vr2 = wp.tile([1, w], F32, name="vr2", tag="r_b", bufs=2)
            nc.vector.scalar_tensor_tensor(vr2[:], sq_ps[:], 1.0 / D,
                                           msq[:], op0=ALU.mult,
                                           op1=ALU.subtract)
            std1 = wp.tile([1, w], F32, name="std1", tag="r_a", bufs=2)
            nc.scalar.activation(std1[:], vr2[:], ACTF.Sqrt,
                                 bias=eps_t[0:1, :])
            inv1 = wp.tile([1, w], F32, name="inv1", tag="r_b", bufs=2)
            nc.vector.reciprocal_approx_fast(inv1[:], std1[:])
            inv1_b = wp.tile([128, w], F32, name="inv1_b", tag="inv1_b",
                             bufs=1, padded_shape=[128, 512])
            nc.gpsimd.partition_broadcast(inv1_b[:], inv1[:])
            # v = (u - mu)*inv1 + x_old ;  second LN has exactly-zero mean
            v = [wp.tile([128, w], F16, name=f"v{ci}_{dc}", tag=f"v_{dc}",
                         bufs=1, padded_shape=[128, 512]) for dc in range(DC)]
            for dc in range(DC):
                d1 = wp.tile([128, w], F16, name="d1", tag="d1", bufs=1,
                             padded_shape=[128, 512])
                nc.vector.tensor_sub(d1[:], u[dc][:], mu_b[:])
                t2 = wp.tile([128, w], F16, name="t2", tag="t2", bufs=1,
                             padded_shape=[128, 512])
                nc.vector.tensor_mul(t2[:], d1[:], inv1_b[:])
                nc.gpsimd.tensor_add(v[dc][:], t2[:],
                                     x_d16h[th][dc][:, l0:l0 + w])
            return v

        def tail_second(layer, ci, v, last=False):
            """d-major tail, part 2: second LN (exactly-zero mean) and the
            x_d16h / x_t16 writes."""
            lo, w, tbs = YM_CHUNKS[ci]
            th = lo // TH
            l0 = lo - TH * th
            sq2_ps = psw(f"sq2_ps_{layer}_{ci}", (1, w))
            for dc in range(DC):
                sq2 = wp.tile([128, w], F16, name="sq2", tag="squ", bufs=2,
                              padded_shape=[128, 512])
                nc.vector.tensor_mul(sq2[:], v[dc][:], v[dc][:])
                nc.tensor.matmul(sq2_ps[:], ones_t[:], sq2[:],
                                 start=(dc == 0), stop=(dc == DC - 1))
            std2 = wp.tile([1, w], F32, name="std2", tag="r_mu", bufs=2)
            nc.scalar.activation(std2[:], sq2_ps[:], ACTF.Sqrt,
                                 bias=eps_t[0:1, :], scale=1.0 / D)
            inv2 = wp.tile([1, w], F32, name="inv2", tag="r_a", bufs=2)
            nc.vector.reciprocal_approx_fast(inv2[:], std2[:])
            inv2_b = wp.tile([128, w], F32, name="inv2_b", tag="inv2_b",
                             bufs=2, padded_shape=[128, 512])
            nc.gpsimd.partition_broadcast(inv2_b[:], inv2[:])
            for dc in range(DC):
                nc.vector.tensor_mul(x_d16h[th][dc][:, l0:l0 + w], v[dc][:],
                                     inv2_b[:])
            # t-major x for the ykv matmuls (off critical path)
            for tb in tbs:
                for dc in range(DC):
                    nc.sync.dma_start_transpose(
                        x_t16[tb][:, 128 * dc:128 * (dc + 1)],
                        x_d16h[th][dc][:, 128 * (tb % 4):
                                       128 * (tb % 4) + 128])
            # lm head for this chunk's token blocks (final layer only)
            if last:
                for tb in tbs:
                    ps_l = psw(f"ps_lg_{tb}", (128, VOCAB))
                    for d in range(DC):
                        nc.tensor.matmul(
                            ps_l[:],
                            x_d16h[th][d][:, 128 * (tb % 4):
                                          128 * (tb % 4) + 128],
                            lmh_sb[d][:], start=(d == 0), stop=(d == DC - 1))
                    lg_sb = wp.tile([128, VOCAB], F32, name="lg_sb",
                                    tag="lg_sb")
                    nc.vector.tensor_copy(lg_sb[:], ps_l[:])
                    nc.sync.dma_start(out_o[128 * tb:128 * (tb + 1), :],
                                      lg_sb[:])

        # ============================================================ layers
        def phaseA_finish(layer, acc):
            spill_A(acc)
            yh0 = sacc("S3", f"S3_y0_{layer}")
            ykv_half0(layer, yh0)

        def emit_rest(layer):
            last = layer == n_layer - 1
            accB = {t: sacc(t, f"{t}_B_{layer}") for t in ("S1", "S2", "S3")}
            emit_phase_th(layer, 1, accB)
            spill_B(accB)
            accC = {"S3": sacc("S3", f"S3_C_{layer}")}
            scores_C(accC)
            spill_C(accC)
            yh1 = sacc("S1", f"S1_y1_{layer}")
            ykv_half1(layer, yh1)

            ib0 = load_stats_half(layer, 0)
            ym_acc = {0: sacc("S3", f"S3_ym_{layer}"),
                      1: sacc("S1", f"S1_ym_{layer}")}
            phase3_pass(layer, 0, ym_acc)
            ym_reduce(layer, 0, ib0, ym_acc)
            ib1 = load_stats_half(layer, 1)
            phase3_pass(layer, 1, ym_acc)
            ym_reduce(layer, 1, ib1, ym_acc)
            v0 = tail_first(layer, 0)
            phase3_pass(layer, 2, ym_acc)
            ym_reduce(layer, 2, ib1, ym_acc)
            tail_second(layer, 0, v0, last=last)
            v1 = tail_first(layer, 1)
            if layer + 1 < n_layer:
                # interleave next layer's PHASE A with the remaining tails
                accA = {t: sacc(t, f"{t}_A_{layer + 1}")
                        for t in ("S1", "S2")}
                emit_phase_th(layer + 1, 0, accA, 0, 8)
                tail_second(layer, 1, v1, last=last)
                emit_phase_th(layer + 1, 0, accA, 8, 12)
                v2 = tail_first(layer, 2)
                tail_second(layer, 2, v2, last=last)
                emit_phase_th(layer + 1, 0, accA, 12, NJ)
                phaseA_finish(layer + 1, accA)
            else:
                tail_second(layer, 1, v1, last=last)
                v2 = tail_first(layer, 2)
                tail_second(layer, 2, v2, last=last)

        accA0 = {t: sacc(t, f"{t}_A_0") for t in ("S1", "S2")}
        emit_phase_th(0, 0, accA0)
        phaseA_finish(0, accA0)
        for layer in range(n_layer):
            emit_rest(layer)

    nc.compile()
    return nc


# ------------------------------------------------------------- host helpers
def _host_tables():
    """cos/sin rope tables in [pair, t] layout, mirroring reference fp32 math."""
    n = np.arange(N, dtype=np.float32)
    q = np.floor(n / 2.0) * 2.0
    freqs = (1.0 / (np.float32(THETA) ** (q / np.float32(N)))
             / np.float32(2.0 * math.pi)).astype(np.float32)
    t = np.arange(T, dtype=np.float32)
    phases = (t[:, None] * freqs[None, :]) % 1.0
    phases = phases * np.float32(2.0 * math.pi)
    cos = np.cos(phases).astype(np.float32)   # [T, N]
    sin = np.sin(phases).astype(np.float32)
    # pair p uses freq of n=2p; table[p, t]
    cos_p = cos[:, 0::2].T.copy()  # [N//2, T]
    sin_p = sin[:, 0::2].T.copy()
    return cos_p, sin_p


def _perm_local():
    """Local latent permutation: position -> (pair index, odd flag)."""
    pos_to_pair = np.empty(NHALF, dtype=np.int64)
    pos_is_odd = np.empty(NHALF, dtype=np.int64)
    for j in range(NJ):
        pr = np.arange(128) + 128 * j
        pos_to_pair[256 * j:256 * j + 128] = pr
        pos_is_odd[256 * j:256 * j + 128] = 0
        pos_to_pair[256 * j + 128:256 * j + 256] = pr
        pos_is_odd[256 * j + 128:256 * j + 256] = 1
    return pos_to_pair, pos_is_odd


_NC_CACHE = {}


def _get_nc():
    if "nc" not in _NC_CACHE:
        _NC_CACHE["nc"] = build_program()
    return _NC_CACHE["nc"]


def prepare_in_maps(idx, embed, encoder, encoder_v, decoder, lm_head):
    idx = np.asarray(idx)
    embed = np.asarray(embed, dtype=np.float32)
    encoder = np.asarray(encoder, dtype=np.float32)
    encoder_v = np.asarray(encoder_v, dtype=np.float32)
    decoder = np.asarray(decoder, dtype=np.float32)
    lm_head = np.asarray(lm_head, dtype=np.float32)

    cos_p, sin_p = _host_tables()
    pos_to_pair, pos_is_odd = _perm_local()

    cmask = (np.arange(128)[:, None] < np.arange(128)[None, :]).astype(np.float16)
    lmh16 = lm_head.astype(np.float16)

    # x0 = LN(embed)[idx] (host-side input prep, fp32 math as in reference)
    mu = embed.mean(axis=-1, keepdims=True)
    var = embed.var(axis=-1, keepdims=True)
    emb_n = (embed - mu) / np.sqrt(var + np.float32(EPS))
    x0 = emb_n[np.asarray(idx).reshape(T)]          # [T, D] f32
    x0_t = x0.astype(np.float16)
    x0_d = x0.T.copy().astype(np.float16)

    in_maps = []
    for c in range(NCORES):
        h, eta = c // 2, c % 2
        pair_g = NPAIR * eta + pos_to_pair          # global pair index
        n_orig = 2 * pair_g + pos_is_odd            # original n within head
        enc_sh = encoder[h][:, n_orig].astype(np.float16)
        encv_sh = encoder_v[h][:, n_orig].astype(np.float16)
        dec_sh = decoder[h * N + n_orig, :].astype(np.float16)
        cos_sh = cos_p[NPAIR * eta:NPAIR * (eta + 1), :].astype(np.float16)
        sin_sh = sin_p[NPAIR * eta:NPAIR * (eta + 1), :].astype(np.float16)
        # [c_th0 | s_th0 | c_th1 | s_th1]
        cos2 = np.concatenate([cos_sh[:, :TH], sin_sh[:, :TH],
                               cos_sh[:, TH:], sin_sh[:, TH:]], axis=1)
        in_maps.append({
            "x0_t": x0_t, "x0_d": x0_d, "enc_sh": enc_sh,
            "encv_sh": encv_sh, "dec_sh": dec_sh, "lmh": lmh16,
            "cos2_sh": cos2, "cmask": cmask,
        })
    return in_maps


def kernel(idx, embed, encoder, encoder_v, decoder, lm_head):
    in_maps = prepare_in_maps(idx, embed, encoder, encoder_v, decoder,
                              lm_head)
    nc = _get_nc()
    res = bass_utils.run_bass_kernel_spmd(nc, in_maps,
                                          core_ids=list(range(NCORES)))
    _NC_CACHE["last_results"] = res
    logits = np.asarray(res.results[0]["logits"], dtype=np.float32)
    return logits.reshape(1, T, VOCAB)


# revision 55
# speedup vs baseline: 1.0408x; 1.0408x over previous
"""Trainium2 Bass kernel for nn_BDH_6313601925221 (sparse_attention).

Model (reference.py):
  x = LN(embed[idx])                                   (B=1, T=1024, D=256)
  repeat 6 layers (shared weights):
    x_sparse = relu(einsum('btd,hdn->bhtn', x, encoder))   N=8192, NH=4
    QR       = rope(x_sparse)                              interleaved-pair rotation
    scores   = einsum('bhtn,bhsn->bhts', QR, QR) * strict_causal
    yKV      = LN(einsum('bhts,bsd->bhtd', scores, x))
    y_sparse = relu(einsum('bhtd,hdn->bhtn', yKV, encoder_v))
    yMLP     = (x_sparse*y_sparse).transpose -> (T, NH*N) @ decoder
    x        = LN(x + LN(yMLP))
  logits = x @ lm_head

Distribution (8 cores): core c = (head h=c//2, latent-half eta=c%2).
Each core computes encoder/rope/scores over its 4096 latent dims.  The
score strips are NEVER exchanged: scores only feed yKV = scores @ x,
which is linear in scores, so each core computes a partial ykvT from its
local strips and a single pairwise AllReduce of ykvT [256,1024] (split
into two t-half chunks) replaces the baseline's four score-strip
AllReduces + DRAM round trip.

The inner LN on yKV is dropped (scale-invariance: relu is positively
homogeneous and the whole path to yMLP is linear in the per-token scale;
x rows are zero-mean so the mean term vanishes).  The deferred 1/std is
applied to the ym partials pre-AllReduce (exact, incl. eps).

Layer schedule (token-half pipelined):
  PHASE A:  th0 encoder+rope j-loop, pipelined A'-scores (kb<4, q<512)
            -> spill strips -> ykv half0 -> pair-AR0        [needs x_d16h th0]
  PHASE B:  th1 encoder+rope j-loop, pipelined B'-scores (kb<4, q>=512)
  PHASE C:  kb>=4 scores sweep -> ykv half1 -> pair-AR1
  pass0 (t 0:512)  -> ym AR ci0;  pass1 (512:768) -> ci1
  TAIL0 (d-major)  -> x_d16h th0 ready
  pass2 (768:1024) -> ci2;  TAIL1
  [NEXT LAYER PHASE A emitted here -- overlaps ci2 AR + TAIL2]
  TAIL2  -> x_d16h th1 complete -> [NEXT LAYER PHASE B ...]

The tail runs fully in d-major layout: column stats via PE-ones matmuls
+ gpsimd partition_broadcast; x_t16 (needed ~50us later by ykv) is
produced by [128,128] transpose DMAs off the critical path.

PSUM (8 banks): S1,S2,S3 = three [128,1024] f32 accumulators (2 banks
each) cycling through score strips / ykv halves / ym accumulation;
tagP = [128,512] bufs=2 (2 banks) for transient matmul outputs.
"""

import math
import sys

import numpy as np

for _p in ("/opt/trn_rl_repo",):
    if _p not in sys.path:
        sys.path.insert(0, _p)

import concourse.bass as bass
import concourse.mybir as mybir
import concourse.tile as tile
from concourse import bacc
from concourse import bass_utils

# ---------------------------------------------------------------- constants
D = 256
NH = 4
N = 8192
T = 1024
N_LAYER = 6
VOCAB = 256
THETA = 2 ** 16
EPS = 1e-5
NCORES = 8

NHALF = N // 2          # 4096 latent dims per core
NPAIR = NHALF // 2      # 2048 rope pairs per core
NT = NHALF // 128       # 32 local n-tiles of 128
NJ = NT // 2            # 16 pair-blocks (tile 2j = evens, 2j+1 = odds)
TB = T // 128           # 8 token blocks
DC = D // 128           # 2 d-chunks
TH = T // 2             # 512 token half

F16 = mybir.dt.float16
F32 = mybir.dt.float32
F8 = mybir.dt.float8e4
I32 = mybir.dt.int32
DR = mybir.MatmulPerfMode.DoubleRow
AX = mybir.AxisListType
ALU = mybir.AluOpType
ACTF = mybir.ActivationFunctionType

ALPHA = 1.0 / 512.0     # yKV pre-scale (overflow headroom; cancels exactly)
BETA = 1.0 / 16.0       # extra scale inside Square so sq fits fp16

# phase-3 / yMLP-AllReduce / tail chunks: (t-col lo, width, token blocks)
YM_CHUNKS = [(0, 512, (0, 1, 2, 3)), (512, 256, (4, 5)), (768, 256, (6, 7))]

# PSUM: S1 [128,1024], S2 [128,512], S3 [128,1024], tagP [128,512]x3
# A' strip PSUM layout: kb -> (S-tag, col offset); strip width (4-kb)*128
A_LAY = {0: ("S1", 0), 1: ("S1", 512), 2: ("S2", 0), 3: ("S2", 256)}
# B-loop strips (kb 0..4, q in [512,1024), width 512 each; kb4 rides along
# in the j-loop to keep it PE-bound)
B_LAY = {0: ("S1", 0), 1: ("S1", 512), 2: ("S3", 0), 3: ("S3", 512),
         4: ("S2", 0)}
# C strips (kb 5..7, width (8-kb)*128)
C_LAY = {5: ("S3", 0), 6: ("S3", 512), 7: ("S3", 896)}


def build_program(n_layer=N_LAYER):
    nc = bacc.Bacc("TRN2", target_bir_lowering=False, debug=False,
                   num_devices=NCORES)

    # ------------------------------------------------------------- I/O decl
    # x0 = LN(embed)[idx] precomputed on host (input prep, like the rope
    # tables); provided in both t-major and d-major layouts.
    x0t_i = nc.dram_tensor("x0_t", [T, D], F16, kind="ExternalInput")
    x0d_i = nc.dram_tensor("x0_d", [D, T], F16, kind="ExternalInput")
    enc_i = nc.dram_tensor("enc_sh", [D, NHALF], F16, kind="ExternalInput")
    encv_i = nc.dram_tensor("encv_sh", [D, NHALF], F16, kind="ExternalInput")
    dec_i = nc.dram_tensor("dec_sh", [NHALF, D], F16, kind="ExternalInput")
    lmh_i = nc.dram_tensor("lmh", [D, VOCAB], F16, kind="ExternalInput")
    # per pair-block row: [c_th0 | s_th0 | c_th1 | s_th1], each TH wide
    cos2_i = nc.dram_tensor("cos2_sh", [NPAIR, 2 * T], F16,
                            kind="ExternalInput")
    cmask_i = nc.dram_tensor("cmask", [128, 128], F16, kind="ExternalInput")
    out_o = nc.dram_tensor("logits", [T, VOCAB], F32, kind="ExternalOutput")

    pair_groups = [[2 * h, 2 * h + 1] for h in range(NH)]
    all_group = [list(range(NCORES))]

    with tile.TileContext(nc) as tc:
      with (
        tc.tile_pool(name="persist", bufs=1) as pp,
        tc.tile_pool(name="work", bufs=2) as wp,
        tc.tile_pool(name="psW", bufs=2, space="PSUM") as psW,
        tc.tile_pool(name="psAcc", bufs=1, space="PSUM") as psAcc,
        tc.tile_pool(name="dram", bufs=1, space="DRAM") as dp,
      ):
        # ------------------------------------------------- persistent SBUF
        enc_sb = [pp.tile([128, NHALF], F16, name=f"enc{d}", tag=f"enc{d}")
                  for d in range(DC)]
        encv_sb = [pp.tile([128, NHALF], F16, name=f"encv{d}", tag=f"encv{d}")
                   for d in range(DC)]
        dec_sb = [pp.tile([128, D], F16, name=f"dec{i}", tag=f"dec{i}")
                  for i in range(NT)]
        # QR stored fp8 (e4m3) in DoubleRow layout: [128, k-subtile, t];
        # subtile 0 = even-parity latent tile (qe), 1 = odd (qo).  The
        # scores matmul runs in fp8 DoubleRow at 0.5 cycles/row -- final
        # error impact measured at ~1.2e-3 (errors average through yKV).
        QR8 = [pp.tile([128, 2, T], F8, name=f"qr8_{p}", tag=f"qr8_{p}")
               for p in range(NJ)]
        # local score strips in SBUF (fp16, diag-masked)
        ST_lo = [pp.tile([128, (4 - kb) * 128], F16, name=f"stl{kb}",
                         tag=f"stl{kb}") for kb in range(4)]
        ST_hi = [pp.tile([128, min(512, (8 - kb) * 128)], F16,
                         name=f"sth{kb}", tag=f"sth{kb}") for kb in range(8)]
        x_t16 = [pp.tile([128, D], F16, name=f"xt16_{i}", tag=f"xt16_{i}")
                 for i in range(TB)]
        x_d16h = [[pp.tile([128, TH], F16, name=f"xd16_{th}_{i}",
                           tag=f"xd16_{th}_{i}") for i in range(DC)]
                  for th in range(2)]
        ykvT = [pp.tile([128, T], F16, name=f"ykvT{i}", tag=f"ykvT{i}")
                for i in range(DC)]
        cmask = pp.tile([128, 128], F16, name="cmaskt", tag="cmaskt")
        eps_t = pp.tile([128, 1], F32, name="eps_t", tag="eps_t")
        ones_t = pp.tile([128, 1], F16, name="ones_t", tag="ones_t")
        eps2_t = pp.tile([1, 1], F32, name="eps2_t", tag="eps2_t")
        lmh_sb = [pp.tile([128, VOCAB], F16, name=f"lmh{d}", tag=f"lmh{d}")
                  for d in range(DC)]

        # ---------------------------------------------------- DRAM buffers
        xs_spill = [dp.tile([NHALF, T], F16, name=f"xs_spill{i}")
                    for i in range(2)]
        ykv_ins = [dp.tile([D, TH], F16, name=f"ykv_in{h}", tag=f"ykv_in{h}")
                   for h in range(2)]
        ykv_outs = [[dp.tile([D, TH], F16, name=f"ykv_out{l}_{h}",
                             tag=f"ykv_out{l}_{h}")
                     for h in range(2)] for l in range(n_layer)]
        ym_ins = [dp.tile([D, w], F16, name=f"ym_in{ci}", tag=f"ym_in{ci}")
                  for ci, (_, w, _) in enumerate(YM_CHUNKS)]
        ym_outs = [[dp.tile([D, w], F16, name=f"ym_out{l}_{ci}",
                            tag=f"ym_out{l}_{ci}", addr_space="Shared")
                    for ci, (_, w, _) in enumerate(YM_CHUNKS)]
                   for l in range(n_layer)]

        def psw(name, shape=(128, 512), dtype=F32):
            return psW.tile(list(shape), dtype, name=name, tag="ps_w",
                            padded_shape=[128, 512], bufs=3)

        def sacc(tag, name):
            w = 512 if tag == "S2" else 1024
            return psAcc.tile([128, w], F32, name=name, tag=tag)

        # ------------------------------------------------------ load consts
        nc.gpsimd.memset(eps_t[:], EPS)
        nc.gpsimd.memset(ones_t[:], 1.0)
        nc.gpsimd.memset(eps2_t[:], EPS * ALPHA * ALPHA)
        nc.sync.dma_start(cmask[:], cmask_i[:, :])
        for i in range(NT):
            nc.scalar.dma_start(dec_sb[i][:], dec_i[128 * i:128 * (i + 1), :])

        # ------------------------------------------------------- x0 loads
        for d in range(DC):
            nc.sync.dma_start(enc_sb[d][:], enc_i[128 * d:128 * (d + 1), :])
            nc.sync.dma_start(encv_sb[d][:],
                              encv_i[128 * d:128 * (d + 1), :])
            nc.sync.dma_start(lmh_sb[d][:], lmh_i[128 * d:128 * (d + 1), :])
        for tb in range(TB):
            nc.sync.dma_start(x_t16[tb][:], x0t_i[128 * tb:128 * (tb + 1), :])
        for th in range(2):
            for d in range(DC):
                nc.sync.dma_start(x_d16h[th][d][:],
                                  x0d_i[128 * d:128 * (d + 1),
                                        TH * th:TH * (th + 1)])

        # ===================================================== layer pieces
        def emit_phase_th(layer, th, acc, j_lo=0, j_hi=NJ):
            """Encoder+rope j-loop segment [j_lo, j_hi) for token half `th`,
            with depth-2 pipelined scores into `acc` (A' strips for th=0,
            B' strips for th=1).  Flushes trailing pairs when j_hi==NJ."""
            xs = xs_spill[layer % 2]
            lay = A_LAY if th == 0 else B_LAY

            def scores_pair(p):
                for kb in range(4 if th == 0 else 5):
                    tag, off = lay[kb]
                    if th == 0:
                        w = (4 - kb) * 128
                        q0 = 128 * kb
                    else:
                        w = 512
                        q0 = 512
                    nc.tensor.matmul(
                        acc[tag][:, off:off + w],
                        QR8[p][:, :, 128 * kb:128 * (kb + 1)],
                        QR8[p][:, :, q0:q0 + w],
                        start=(p == 0), stop=(p == NJ - 1),
                        perf_mode=DR)

            for j in range(j_lo, j_hi):
                cs2 = wp.tile([128, T], F16, name="cs2", tag="cs2", bufs=4)
                nc.sync.dma_start(cs2[:],
                                  cos2_i[128 * j:128 * (j + 1),
                                         T * th:T * (th + 1)])
                xs2 = wp.tile([128, T], F16, name="xs2", tag="xs2", bufs=5)
                for par in range(2):
                    nt = 2 * j + par
                    ps_e = psw(f"ps_enc_{layer}_{th}_{nt}")
                    for d in range(DC):
                        nc.tensor.matmul(
                            ps_e[:],
                            enc_sb[d][:, 128 * nt:128 * (nt + 1)],
                            x_d16h[th][d][:],
                            start=(d == 0), stop=(d == DC - 1))
                    nc.scalar.activation(xs2[:, TH * par:TH * (par + 1)],
                                         ps_e[:], ACTF.Relu)
                nc.sync.dma_start(
                    xs[256 * j:256 * (j + 1),
                       TH * th:TH * (th + 1)].rearrange(
                        "(b p) n -> p b n", p=128),
                    xs2[:].rearrange("p (b n) -> p b n", n=TH))
                # rope: cs2 = [c|s], xs2 = [xe|xo]; m2 split DVE/GpSimd to
                # balance engine load (j-loop is rope-throughput paced).
                # qe/qo land in fp16 (1-byte writes are slow on DVE/GpSimd);
                # one wide ACT copy converts both into the fp8 DR layout.
                m1 = wp.tile([128, T], F16, name="m1", tag="rope_m", bufs=6)
                nc.vector.tensor_mul(m1[:], xs2[:], cs2[:])
                m3 = wp.tile([128, T], F16, name="m3", tag="rope_q", bufs=6)
                nc.vector.tensor_sub(m3[:, 0:TH], m1[:, 0:TH], m1[:, TH:T])
                m2 = wp.tile([128, T], F16, name="m2", tag="rope_m", bufs=6)
                nc.gpsimd.tensor_mul(m2[:, 0:TH], xs2[:, TH:T], cs2[:, 0:TH])
                nc.vector.tensor_mul(m2[:, TH:T], xs2[:, 0:TH], cs2[:, TH:T])
                nc.gpsimd.tensor_add(m3[:, TH:T], m2[:, 0:TH], m2[:, TH:T])
                nc.scalar.activation(
                    QR8[j][:, :, TH * th:TH * (th + 1)],
                    m3[:].rearrange("p (s n) -> p s n", s=2),
                    ACTF.Copy)
                if j >= 4:
                    scores_pair(j - 4)
            if j_hi == NJ:
                for p in (NJ - 4, NJ - 3, NJ - 2, NJ - 1):
                    scores_pair(p)

        def spill_A(acc):
            for kb in range(4):
                tag, off = A_LAY[kb]
                w = (4 - kb) * 128
                nc.vector.tensor_copy(ST_lo[kb][:], acc[tag][:, off:off + w])
                nc.gpsimd.tensor_mul(ST_lo[kb][:, 0:128],
                                     ST_lo[kb][:, 0:128], cmask[:])

        def spill_B(acc):
            for kb in range(5):
                tag, off = B_LAY[kb]
                nc.vector.tensor_copy(ST_hi[kb][:], acc[tag][:, off:off + 512])
            nc.gpsimd.tensor_mul(ST_hi[4][:, 0:128],
                                 ST_hi[4][:, 0:128], cmask[:])

        def spill_C(acc):
            for kb in range(5, 8):
                tag, off = C_LAY[kb]
                w = (8 - kb) * 128
                nc.vector.tensor_copy(ST_hi[kb][:], acc[tag][:, off:off + w])
                nc.gpsimd.tensor_mul(ST_hi[kb][:, 0:128],
                                     ST_hi[kb][:, 0:128], cmask[:])

        def scores_C(acc):
            for p in range(NJ):
                for kb in range(5, 8):
                    tag, off = C_LAY[kb]
                    w = (8 - kb) * 128
                    nc.tensor.matmul(
                        acc[tag][:, off:off + w],
                        QR8[p][:, :, 128 * kb:128 * (kb + 1)],
                        QR8[p][:, :, 128 * kb:128 * kb + w],
                        start=(p == 0), stop=(p == NJ - 1),
                        perf_mode=DR)

        def ykv_half0(layer, yh):
            # ykvT partial, cols q in [0,512): strips kb 0..3 (ST_lo)
            for dc in range(DC):
                for kb in range(4):
                    w = (4 - kb) * 128
                    nc.tensor.matmul(
                        yh[:, 512 * dc + 128 * kb:512 * dc + 512],
                        x_t16[kb][:, 128 * dc:128 * (dc + 1)],
                        ST_lo[kb][:, 0:w],
                        start=(kb == 0), stop=(kb == 3))
            for dc in range(DC):
                yk = wp.tile([128, TH], F16, name="yk0", tag="yk_sb", bufs=2)
                nc.scalar.mul(yk[:], yh[:, 512 * dc:512 * dc + 512], ALPHA)
                nc.scalar.dma_start(ykv_ins[0][128 * dc:128 * (dc + 1), :],
                                    yk[:])
            nc.gpsimd.collective_compute(
                "AllReduce", ALU.add, replica_groups=pair_groups,
                ins=[ykv_ins[0].opt()], outs=[ykv_outs[layer][0].opt()])

        def ykv_half1(layer, yh):
            # ykvT partial, cols q in [512,1024): strips kb 0..7 (ST_hi)
            for dc in range(DC):
                for kb in range(8):
                    a = max(512, 128 * kb)
                    w = 1024 - a
                    nc.tensor.matmul(
                        yh[:, 512 * dc + a - 512:512 * dc + 512],
                        x_t16[kb][:, 128 * dc:128 * (dc + 1)],
                        ST_hi[kb][:, 0:w],
                        start=(kb == 0), stop=(kb == 7))
            for dc in range(DC):
                yk = wp.tile([128, TH], F16, name="yk1", tag="yk_sb", bufs=2)
                nc.scalar.mul(yk[:], yh[:, 512 * dc:512 * dc + 512], ALPHA)
                nc.scalar.dma_start(ykv_ins[1][128 * dc:128 * (dc + 1), :],
                                    yk[:])
            nc.gpsimd.collective_compute(
                "AllReduce", ALU.add, replica_groups=pair_groups,
                ins=[ykv_ins[1].opt()], outs=[ykv_outs[layer][1].opt()])

        def load_stats_half(layer, h):
            # load reduced ykvT half and compute per-token 1/std (deferred
            # inner LayerNorm; rows zero-mean so var = E[y^2])
            for dc in range(DC):
                nc.sync.dma_start(
                    ykvT[dc][:, TH * h:TH * (h + 1)],
                    ykv_outs[layer][h][128 * dc:128 * (dc + 1), :])
            ssq_ps = psW.tile([1, 512], F32, name=f"ssq_{layer}_{h}",
                              tag="ps_w", padded_shape=[128, 512], bufs=3)
            for dc in range(DC):
                sqt = wp.tile([128, TH], F16, name="sqt", tag="sqt", bufs=1)
                nc.scalar.activation(sqt[:], ykvT[dc][:, TH * h:TH * (h + 1)],
                                     ACTF.Square, scale=BETA)
                nc.tensor.matmul(ssq_ps[:], ones_t[:], sqt[:],
                                 start=(dc == 0), stop=(dc == DC - 1))
            std_row = wp.tile([1, TH], F32, name="std_row", tag="r_a",
                              bufs=2)
            nc.scalar.activation(std_row[:], ssq_ps[:], ACTF.Sqrt,
                                 bias=eps2_t[:],
                                 scale=1.0 / (D * BETA * BETA))
            inv_row = wp.tile([1, TH], F32, name="inv_row", tag="r_b",
                              bufs=2)
            nc.vector.reciprocal_approx_fast(inv_row[:], std_row[:])
            ib = wp.tile([128, TH], F32, name=f"inv_b{h}", tag=f"inv_b{h}",
                         bufs=1)
            nc.gpsimd.partition_broadcast(ib[:], inv_row[:])
            return ib

        def phase3_pass(layer, ci, ym_acc):
            lo, w, _ = YM_CHUNKS[ci]
            xs = xs_spill[layer % 2]
            prev = None
            xs_c2 = None

            def emit_ym(nt, xy):
                for dh in range(DC):
                    nc.tensor.matmul(
                        ym_acc[dh][:, lo:lo + w],
                        dec_sb[nt][:, 128 * dh:128 * (dh + 1)],
                        xy[:], start=(nt == 0), stop=(nt == NT - 1))

            for nt in range(NT):
                ps_v = psw(f"ps_ysp_{layer}_{nt}_{ci}", (128, w))
                for d in range(DC):
                    nc.tensor.matmul(
                        ps_v[:], encv_sb[d][:, 128 * nt:128 * (nt + 1)],
                        ykvT[d][:, lo:lo + w],
                        start=(d == 0), stop=(d == DC - 1))
                if nt % 2 == 0:
                    xs_c2 = wp.tile([128, 2 * w], F16, name="xs_c",
                                    tag="xs_c", bufs=3,
                                    padded_shape=[128, 1024])
                    nc.sync.dma_start(
                        xs_c2[:].rearrange("p (b n) -> p b n", n=w),
                        xs[128 * nt:128 * (nt + 2), lo:lo + w].rearrange(
                            "(b p) n -> p b n", p=128))
                xs_ap = xs_c2[:, w * (nt % 2):w * (nt % 2) + w]
                xy = wp.tile([128, w], F16, name="xy", tag="xy", bufs=4,
                             padded_shape=[128, 512])
                if ci == 0 and nt % 2 == 1:
                    # offload every other xy: relu on ACT (PSUM-capable),
                    # multiply on GpSimd -- DVE is otherwise the pacer
                    ys = wp.tile([128, w], F16, name="ys", tag="ys", bufs=2,
                                 padded_shape=[128, 512])
                    nc.scalar.activation(ys[:], ps_v[:], ACTF.Relu)
                    nc.gpsimd.tensor_mul(xy[:], ys[:], xs_ap)
                else:
                    nc.vector.scalar_tensor_tensor(
                        xy[:], ps_v[:], 0.0, xs_ap, op0=ALU.max, op1=ALU.mult)
                if prev is not None:
                    emit_ym(*prev)
                prev = (nt, xy)
            emit_ym(*prev)

        def ym_reduce(layer, ci, ib, ym_acc):
            lo, w, _ = YM_CHUNKS[ci]
            for dh in range(DC):
                ym_sb = wp.tile([128, w], F16, name=f"ym_sb{ci}",
                                tag="ym_sb", bufs=2, padded_shape=[128, 512])
                nc.vector.tensor_mul(ym_sb[:], ym_acc[dh][:, lo:lo + w],
                                     ib[:, lo % 512:lo % 512 + w])
                nc.scalar.dma_start(ym_ins[ci][128 * dh:128 * (dh + 1), :],
                                    ym_sb[:])
            nc.gpsimd.collective_compute(
                "AllReduce", ALU.add, replica_groups=all_group,
                ins=[ym_ins[ci].opt()], outs=[ym_outs[layer][ci].opt()])

        def tail_first(layer, ci):
            """d-major tail, part 1: load u, column stats (PE-ones), first
            LN chain up to v = (u - mu)*inv1 + x_old.  No PE work after the
            mu/sq matmuls, so subsequent PE instructions (pass2 etc.) are
            not blocked behind the DVE latency chain."""
            lo, w, tbs = YM_CHUNKS[ci]
            th = lo // TH
            l0 = lo - TH * th
            ym_out = ym_outs[layer][ci]
            u = [wp.tile([128, w], F16, name=f"u{ci}_{dc}", tag=f"u_{dc}",
                         bufs=1, padded_shape=[128, 512]) for dc in range(DC)]
            for dc in range(DC):
                nc.sync.dma_start(u[dc][:],
                                  ym_out[128 * dc:128 * (dc + 1), :])
            # column stats over d (partition dim) via PE-ones matmuls
            mu_ps = psw(f"mu_ps_{layer}_{ci}", (1, w))
            for dc in range(DC):
                nc.tensor.matmul(mu_ps[:], ones_t[:], u[dc][:],
                                 start=(dc == 0), stop=(dc == DC - 1))
            sq_ps = psw(f"sq_ps_{layer}_{ci}", (1, w))
            for dc in range(DC):
                squ = wp.tile([128, w], F16, name="squ", tag="squ", bufs=2,
                              padded_shape=[128, 512])
                nc.scalar.activation(squ[:], u[dc][:], ACTF.Square)
                nc.tensor.matmul(sq_ps[:], ones_t[:], squ[:],
                                 start=(dc == 0), stop=(dc == DC - 1))
            mu_row = wp.tile([1, w], F32, name="mu_row", tag="r_mu", bufs=2)
            nc.scalar.mul(mu_row[:], mu_ps[:], 1.0 / D)
            mu_b = wp.tile([128, w], F32, name="mu_b", tag="mu_b", bufs=1,
                           padded_shape=[128, 512])
            nc.gpsimd.partition_broadcast(mu_b[:], mu_row[:])
            msq = wp.tile([1, w], F32, name="msq", tag="r_a", bufs=2)
            nc.vector.tensor_mul(msq[:], mu_row[:], mu_row[:])
            # vr2 = ssq/D - mu^2  (one fused DVE op, PSUM source)
            vr2 = wp.tile([1, w], F32, name="vr2", tag="r_b", bufs=2)
            nc.vector.scalar_tensor_tensor(vr2[:], sq_ps[:], 1.0 / D,
                                           msq[:], op0=ALU.mult,
                                           op1=ALU.subtract)
            std1 = wp.tile([1, w], F32, name="std1", tag="r_a", bufs=2)
            nc.scalar.activation(std1[:], vr2[:], ACTF.Sqrt,
                                 bias=eps_t[0:1, :])
            inv1 = wp.tile([1, w], F32, name="inv1", tag="r_b", bufs=2)
            nc.vector.reciprocal_approx_fast(inv1[:], std1[:])
            inv1_b = wp.tile([128, w], F32, name="inv1_b", tag="inv1_b",
                             bufs=1, padded_shape=[128, 512])
            nc.gpsimd.partition_broadcast(inv1_b[:], inv1[:])
            # v = (u - mu)*inv1 + x_old ;  second LN has exactly-zero mean
            v = [wp.tile([128, w], F16, name=f"v{ci}_{dc}", tag=f"v_{dc}",
                         bufs=1, padded_shape=[128, 512]) for dc in range(DC)]
            for dc in range(DC):
                d1 = wp.tile([128, w], F16, name="d1", tag="d1", bufs=1,
                             padded_shape=[128, 512])
                nc.vector.tensor_sub(d1[:], u[dc][:], mu_b[:])
                t2 = wp.tile([128, w], F16, name="t2", tag="t2", bufs=1,
                             padded_shape=[128, 512])
                nc.vector.tensor_mul(t2[:], d1[:], inv1_b[:])
                nc.gpsimd.tensor_add(v[dc][:], t2[:],
                                     x_d16h[th][dc][:, l0:l0 + w])
            return v

        def tail_second(layer, ci, v, last=False):
            """d-major tail, part 2: second LN (exactly-zero mean) and the
            x_d16h / x_t16 writes."""
            lo, w, tbs = YM_CHUNKS[ci]
            th = lo // TH
            l0 = lo - TH * th
            sq2_ps = psw(f"sq2_ps_{layer}_{ci}", (1, w))
            for dc in range(DC):
                sq2 = wp.tile([128, w], F16, name="sq2", tag="squ", bufs=2,
                              padded_shape=[128, 512])
                nc.scalar.activation(sq2[:], v[dc][:], ACTF.Square)
                nc.tensor.matmul(sq2_ps[:], ones_t[:], sq2[:],
                                 start=(dc == 0), stop=(dc == DC - 1))
            std2 = wp.tile([1, w], F32, name="std2", tag="r_mu", bufs=2)
            nc.scalar.activation(std2[:], sq2_ps[:], ACTF.Sqrt,
                                 bias=eps_t[0:1, :], scale=1.0 / D)
            inv2 = wp.tile([1, w], F32, name="inv2", tag="r_a", bufs=2)
            nc.vector.reciprocal_approx_fast(inv2[:], std2[:])
            inv2_b = wp.tile([128, w], F32, name="inv2_b", tag="inv2_b",
                             bufs=2, padded_shape=[128, 512])
            nc.gpsimd.partition_broadcast(inv2_b[:], inv2[:])
            for dc in range(DC):
                nc.vector.tensor_mul(x_d16h[th][dc][:, l0:l0 + w], v[dc][:],
                                     inv2_b[:])
            # t-major x for the ykv matmuls (off critical path)
            for tb in tbs:
                for dc in range(DC):
                    nc.sync.dma_start_transpose(
                        x_t16[tb][:, 128 * dc:128 * (dc + 1)],
                        x_d16h[th][dc][:, 128 * (tb % 4):
                                       128 * (tb % 4) + 128])
            # lm head for this chunk's token blocks (final layer only)
            if last:
                for tb in tbs:
                    ps_l = psw(f"ps_lg_{tb}", (128, VOCAB))
                    for d in range(DC):
                        nc.tensor.matmul(
                            ps_l[:],
                            x_d16h[th][d][:, 128 * (tb % 4):
                                          128 * (tb % 4) + 128],
                            lmh_sb[d][:], start=(d == 0), stop=(d == DC - 1))
                    lg_sb = wp.tile([128, VOCAB], F32, name="lg_sb",
                                    tag="lg_sb")
                    nc.vector.tensor_copy(lg_sb[:], ps_l[:])
                    nc.sync.dma_start(out_o[128 * tb:128 * (tb + 1), :],
                                      lg_sb[:])

        # ============================================================ layers
        def phaseA_finish(layer, acc):
            spill_A(acc)
            yh0 = sacc("S3", f"S3_y0_{layer}")
            ykv_half0(layer, yh0)

        def emit_rest(layer):
            last = layer == n_layer - 1
            accB = {t: sacc(t, f"{t}_B_{layer}") for t in ("S1", "S2", "S3")}
            emit_phase_th(layer, 1, accB)
            spill_B(accB)
            accC = {"S3": sacc("S3", f"S3_C_{layer}")}
            scores_C(accC)
            spill_C(accC)
            yh1 = sacc("S1", f"S1_y1_{layer}")
            ykv_half1(layer, yh1)

            ib0 = load_stats_half(layer, 0)
            ym_acc = {0: sacc("S3", f"S3_ym_{layer}"),
                      1: sacc("S1", f"S1_ym_{layer}")}
            phase3_pass(layer, 0, ym_acc)
            ym_reduce(layer, 0, ib0, ym_acc)
            ib1 = load_stats_half(layer, 1)
            phase3_pass(layer, 1, ym_acc)
            ym_reduce(layer, 1, ib1, ym_acc)
            v0 = tail_first(layer, 0)
            phase3_pass(layer, 2, ym_acc)
            ym_reduce(layer, 2, ib1, ym_acc)
            tail_second(layer, 0, v0, last=last)
            v1 = tail_first(layer, 1)
            if layer + 1 < n_layer:
                # interleave next layer's PHASE A with the remaining tails
                accA = {t: sacc(t, f"{t}_A_{layer + 1}")
                        for t in ("S1", "S2")}
                emit_phase_th(layer + 1, 0, accA, 0, 8)
                tail_second(layer, 1, v1, last=last)
                emit_phase_th(layer + 1, 0, accA, 8, 12)
                v2 = tail_first(layer, 2)
                tail_second(layer, 2, v2, last=last)
                emit_phase_th(layer + 1, 0, accA, 12, NJ)
                phaseA_finish(layer + 1, accA)
            else:
                tail_second(layer, 1, v1, last=last)
                v2 = tail_first(layer, 2)
                tail_second(layer, 2, v2, last=last)

        accA0 = {t: sacc(t, f"{t}_A_0") for t in ("S1", "S2")}
        emit_phase_th(0, 0, accA0)
        phaseA_finish(0, accA0)
        for layer in range(n_layer):
            emit_rest(layer)

    nc.compile()
    return nc


# ------------------------------------------------------------- host helpers
def _host_tables():
    """cos/sin rope tables in [pair, t] layout, mirroring reference fp32 math."""
    n = np.arange(N, dtype=np.float32)
    q = np.floor(n / 2.0) * 2.0
    freqs = (1.0 / (np.float32(THETA) ** (q / np.float32(N)))
             / np.float32(2.0 * math.pi)).astype(np.float32)
    t = np.arange(T, dtype=np.float32)
    phases = (t[:, None] * freqs[None, :]) % 1.0
    phases = phases * np.float32(2.0 * math.pi)
    cos = np.cos(phases).astype(np.float32)   # [T, N]
    sin = np.sin(phases).astype(np.float32)
    # pair p uses freq of n=2p; table[p, t]
    cos_p = cos[:, 0::2].T.copy()  # [N//2, T]
    sin_p = sin[:, 0::2].T.copy()
    return cos_p, sin_p


def _perm_local():
    """Local latent permutation: position -> (pair index, odd flag)."""
    pos_to_pair = np.empty(NHALF, dtype=np.int64)
    pos_is_odd = np.empty(NHALF, dtype=np.int64)
    for j in range(NJ):
        pr = np.arange(128) + 128 * j
        pos_to_pair[256 * j:256 * j + 128] = pr
        pos_is_odd[256 * j:256 * j + 128] = 0
        pos_to_pair[256 * j + 128:256 * j + 256] = pr
        pos_is_odd[256 * j + 128:256 * j + 256] = 1
    return pos_to_pair, pos_is_odd


_NC_CACHE = {}


def _get_nc():
    if "nc" not in _NC_CACHE:
        _NC_CACHE["nc"] = build_program()
    return _NC_CACHE["nc"]


def prepare_in_maps(idx, embed, encoder, encoder_v, decoder, lm_head):
    idx = np.asarray(idx)
    embed = np.asarray(embed, dtype=np.float32)
    encoder = np.asarray(encoder, dtype=np.float32)
    encoder_v = np.asarray(encoder_v, dtype=np.float32)
    decoder = np.asarray(decoder, dtype=np.float32)
    lm_head = np.asarray(lm_head, dtype=np.float32)

    cos_p, sin_p = _host_tables()
    pos_to_pair, pos_is_odd = _perm_local()

    cmask = (np.arange(128)[:, None] < np.arange(128)[None, :]).astype(np.float16)
    lmh16 = lm_head.astype(np.float16)

    # x0 = LN(embed)[idx] (host-side input prep, fp32 math as in reference)
    mu = embed.mean(axis=-1, keepdims=True)
    var = embed.var(axis=-1, keepdims=True)
    emb_n = (embed - mu) / np.sqrt(var + np.float32(EPS))
    x0 = emb_n[np.asarray(idx).reshape(T)]          # [T, D] f32
    x0_t = x0.astype(np.float16)
    x0_d = x0.T.copy().astype(np.float16)

    in_maps = []
    for c in range(NCORES):
        h, eta = c // 2, c % 2
        pair_g = NPAIR * eta + pos_to_pair          # global pair index
        n_orig = 2 * pair_g + pos_is_odd            # original n within head
        enc_sh = encoder[h][:, n_orig].astype(np.float16)
        encv_sh = encoder_v[h][:, n_orig].astype(np.float16)
        dec_sh = decoder[h * N + n_orig, :].astype(np.float16)
        cos_sh = cos_p[NPAIR * eta:NPAIR * (eta + 1), :].astype(np.float16)
        sin_sh = sin_p[NPAIR * eta:NPAIR * (eta + 1), :].astype(np.float16)
        # [c_th0 | s_th0 | c_th1 | s_th1]
        cos2 = np.concatenate([cos_sh[:, :TH], sin_sh[:, :TH],
                               cos_sh[:, TH:], sin_sh[:, TH:]], axis=1)
        in_maps.append({
            "x0_t": x0_t, "x0_d": x0_d, "enc_sh": enc_sh,
            "encv_sh": encv_sh, "dec_sh": dec_sh, "lmh": lmh16,
            "cos2_sh": cos2, "cmask": cmask,
        })
    return in_maps


def kernel(idx, embed, encoder, encoder_v, decoder, lm_head):
    in_maps = prepare_in_maps(idx, embed, encoder, encoder_v, decoder,
                              lm_head)
    nc = _get_nc()
    res = bass_utils.run_bass_kernel_spmd(nc, in_maps,
                                          core_ids=list(range(NCORES)))
    _NC_CACHE["last_results"] = res
    logits = np.asarray(res.results[0]["logits"], dtype=np.float32)
    return logits.reshape(1, T, VOCAB)


# revision 56
# speedup vs baseline: 1.0638x; 1.0221x over previous
"""Trainium2 Bass kernel for nn_BDH_6313601925221 (sparse_attention).

Model (reference.py):
  x = LN(embed[idx])                                   (B=1, T=1024, D=256)
  repeat 6 layers (shared weights):
    x_sparse = relu(einsum('btd,hdn->bhtn', x, encoder))   N=8192, NH=4
    QR       = rope(x_sparse)                              interleaved-pair rotation
    scores   = einsum('bhtn,bhsn->bhts', QR, QR) * strict_causal
    yKV      = LN(einsum('bhts,bsd->bhtd', scores, x))
    y_sparse = relu(einsum('bhtd,hdn->bhtn', yKV, encoder_v))
    yMLP     = (x_sparse*y_sparse).transpose -> (T, NH*N) @ decoder
    x        = LN(x + LN(yMLP))
  logits = x @ lm_head

Distribution (8 cores): core c = (head h=c//2, latent-half eta=c%2).
Each core computes encoder/rope/scores over its 4096 latent dims.  The
score strips are NEVER exchanged: scores only feed yKV = scores @ x,
which is linear in scores, so each core computes a partial ykvT from its
local strips and a single pairwise AllReduce of ykvT [256,1024] (split
into two t-half chunks) replaces the baseline's four score-strip
AllReduces + DRAM round trip.

The inner LN on yKV is dropped (scale-invariance: relu is positively
homogeneous and the whole path to yMLP is linear in the per-token scale;
x rows are zero-mean so the mean term vanishes).  The deferred 1/std is
applied to the ym partials pre-AllReduce (exact, incl. eps).

Layer schedule (token-half pipelined):
  PHASE A:  th0 encoder+rope j-loop, pipelined A'-scores (kb<4, q<512)
            -> spill strips -> ykv half0 -> pair-AR0        [needs x_d16h th0]
  PHASE B:  th1 encoder+rope j-loop, pipelined B'-scores (kb<4, q>=512)
  PHASE C:  kb>=4 scores sweep -> ykv half1 -> pair-AR1
  pass0 (t 0:512)  -> ym AR ci0;  pass1 (512:768) -> ci1
  TAIL0 (d-major)  -> x_d16h th0 ready
  pass2 (768:1024) -> ci2;  TAIL1
  [NEXT LAYER PHASE A emitted here -- overlaps ci2 AR + TAIL2]
  TAIL2  -> x_d16h th1 complete -> [NEXT LAYER PHASE B ...]

The tail runs fully in d-major layout: column stats via PE-ones matmuls
+ gpsimd partition_broadcast; x_t16 (needed ~50us later by ykv) is
produced by [128,128] transpose DMAs off the critical path.

PSUM (8 banks): S1,S2,S3 = three [128,1024] f32 accumulators (2 banks
each) cycling through score strips / ykv halves / ym accumulation;
tagP = [128,512] bufs=2 (2 banks) for transient matmul outputs.
"""

import math
import sys

import numpy as np

for _p in ("/opt/trn_rl_repo",):
    if _p not in sys.path:
        sys.path.insert(0, _p)

import concourse.bass as bass
import concourse.mybir as mybir
import concourse.tile as tile
from concourse import bacc
from concourse import bass_utils

# ---------------------------------------------------------------- constants
D = 256
NH = 4
N = 8192
T = 1024
N_LAYER = 6
VOCAB = 256
THETA = 2 ** 16
EPS = 1e-5
NCORES = 8

NHALF = N // 2          # 4096 latent dims per core
NPAIR = NHALF // 2      # 2048 rope pairs per core
NT = NHALF // 128       # 32 local n-tiles of 128
NJ = NT // 2            # 16 pair-blocks (tile 2j = evens, 2j+1 = odds)
TB = T // 128           # 8 token blocks
DC = D // 128           # 2 d-chunks
TH = T // 2             # 512 token half

F16 = mybir.dt.float16
F32 = mybir.dt.float32
F8 = mybir.dt.float8e4
I32 = mybir.dt.int32
DR = mybir.MatmulPerfMode.DoubleRow
AX = mybir.AxisListType
ALU = mybir.AluOpType
ACTF = mybir.ActivationFunctionType

ALPHA = 1.0 / 512.0     # yKV pre-scale (overflow headroom; cancels exactly)
BETA = 1.0 / 16.0       # extra scale inside Square so sq fits fp16

# phase-3 / yMLP-AllReduce / tail chunks: (t-col lo, width, token blocks)
YM_CHUNKS = [(0, 512, (0, 1, 2, 3)), (512, 256, (4, 5)), (768, 256, (6, 7))]

# PSUM: S1 [128,1024], S2 [128,512], S3 [128,1024], tagP [128,512]x3
# A' strip PSUM layout: kb -> (S-tag, col offset); strip width (4-kb)*128
A_LAY = {0: ("S1", 0), 1: ("S1", 512), 2: ("S2", 0), 3: ("S2", 256)}
# B-loop strips (kb 0..4, q in [512,1024), width 512 each; kb4 rides along
# in the j-loop to keep it PE-bound)
B_LAY = {0: ("S1", 0), 1: ("S1", 512), 2: ("S3", 0), 3: ("S3", 512),
         4: ("S2", 0)}
# C strips (kb 5..7, width (8-kb)*128)
C_LAY = {5: ("S3", 0), 6: ("S3", 512), 7: ("S3", 896)}


def build_program(n_layer=N_LAYER):
    nc = bacc.Bacc("TRN2", target_bir_lowering=False, debug=False,
                   num_devices=NCORES)

    # ------------------------------------------------------------- I/O decl
    # x0 = LN(embed)[idx] precomputed on host (input prep, like the rope
    # tables); provided in both t-major and d-major layouts.
    x0t_i = nc.dram_tensor("x0_t", [T, D], F16, kind="ExternalInput")
    x0d_i = nc.dram_tensor("x0_d", [D, T], F16, kind="ExternalInput")
    enc_i = nc.dram_tensor("enc_sh", [D, NHALF], F16, kind="ExternalInput")
    encv_i = nc.dram_tensor("encv_sh", [D, NHALF], F16, kind="ExternalInput")
    dec_i = nc.dram_tensor("dec_sh", [NHALF, D], F16, kind="ExternalInput")
    lmh_i = nc.dram_tensor("lmh", [D, VOCAB], F16, kind="ExternalInput")
    # per pair-block row: [c_th0 | s_th0 | c_th1 | s_th1], each TH wide
    cos2_i = nc.dram_tensor("cos2_sh", [NPAIR, 2 * T], F16,
                            kind="ExternalInput")
    cmask_i = nc.dram_tensor("cmask", [128, 128], F16, kind="ExternalInput")
    out_o = nc.dram_tensor("logits", [T, VOCAB], F32, kind="ExternalOutput")

    pair_groups = [[2 * h, 2 * h + 1] for h in range(NH)]
    all_group = [list(range(NCORES))]

    with tile.TileContext(nc) as tc:
      with (
        tc.tile_pool(name="persist", bufs=1) as pp,
        tc.tile_pool(name="work", bufs=2) as wp,
        tc.tile_pool(name="psW", bufs=2, space="PSUM") as psW,
        tc.tile_pool(name="psAcc", bufs=1, space="PSUM") as psAcc,
        tc.tile_pool(name="dram", bufs=1, space="DRAM") as dp,
      ):
        # ------------------------------------------------- persistent SBUF
        enc_sb = [pp.tile([128, NHALF], F16, name=f"enc{d}", tag=f"enc{d}")
                  for d in range(DC)]
        encv_sb = [pp.tile([128, NHALF], F16, name=f"encv{d}", tag=f"encv{d}")
                   for d in range(DC)]
        dec_sb = [pp.tile([128, D], F16, name=f"dec{i}", tag=f"dec{i}")
                  for i in range(NT)]
        # QR stored fp8 (e4m3) in DoubleRow layout: [128, k-subtile, t];
        # subtile 0 = even-parity latent tile (qe), 1 = odd (qo).  The
        # scores matmul runs in fp8 DoubleRow at 0.5 cycles/row -- final
        # error impact measured at ~1.2e-3 (errors average through yKV).
        QR8 = [pp.tile([128, 2, T], F8, name=f"qr8_{p}", tag=f"qr8_{p}")
               for p in range(NJ)]
        # local score strips in SBUF (fp16, diag-masked)
        ST_lo = [pp.tile([128, (4 - kb) * 128], F16, name=f"stl{kb}",
                         tag=f"stl{kb}") for kb in range(4)]
        ST_hi = [pp.tile([128, min(512, (8 - kb) * 128)], F16,
                         name=f"sth{kb}", tag=f"sth{kb}") for kb in range(8)]
        x_t16 = [pp.tile([128, D], F16, name=f"xt16_{i}", tag=f"xt16_{i}")
                 for i in range(TB)]
        x_d16h = [[pp.tile([128, TH], F16, name=f"xd16_{th}_{i}",
                           tag=f"xd16_{th}_{i}") for i in range(DC)]
                  for th in range(2)]
        ykvT = [pp.tile([128, T], F16, name=f"ykvT{i}", tag=f"ykvT{i}")
                for i in range(DC)]
        cmask = pp.tile([128, 128], F16, name="cmaskt", tag="cmaskt")
        eps_t = pp.tile([128, 1], F32, name="eps_t", tag="eps_t")
        ones_t = pp.tile([128, 1], F16, name="ones_t", tag="ones_t")
        eps2_t = pp.tile([1, 1], F32, name="eps2_t", tag="eps2_t")
        lmh_sb = [pp.tile([128, VOCAB], F16, name=f"lmh{d}", tag=f"lmh{d}")
                  for d in range(DC)]

        # ---------------------------------------------------- DRAM buffers
        xs_spill = [dp.tile([NHALF, T], F16, name=f"xs_spill{i}")
                    for i in range(2)]
        ykv_ins = [dp.tile([D, TH], F16, name=f"ykv_in{h}", tag=f"ykv_in{h}")
                   for h in range(2)]
        ykv_outs = [[dp.tile([D, TH], F16, name=f"ykv_out{l}_{h}",
                             tag=f"ykv_out{l}_{h}")
                     for h in range(2)] for l in range(n_layer)]
        ym_ins = [dp.tile([D, w], F16, name=f"ym_in{ci}", tag=f"ym_in{ci}")
                  for ci, (_, w, _) in enumerate(YM_CHUNKS)]
        ym_outs = [[dp.tile([D, w], F16, name=f"ym_out{l}_{ci}",
                            tag=f"ym_out{l}_{ci}", addr_space="Shared")
                    for ci, (_, w, _) in enumerate(YM_CHUNKS)]
                   for l in range(n_layer)]

        def psw(name, shape=(128, 512), dtype=F32):
            return psW.tile(list(shape), dtype, name=name, tag="ps_w",
                            padded_shape=[128, 512], bufs=3)

        def sacc(tag, name):
            w = 512 if tag == "S2" else 1024
            return psAcc.tile([128, w], F32, name=name, tag=tag)

        # ------------------------------------------------------ load consts
        nc.gpsimd.memset(eps_t[:], EPS)
        nc.gpsimd.memset(ones_t[:], 1.0)
        nc.gpsimd.memset(eps2_t[:], EPS * ALPHA * ALPHA)
        nc.sync.dma_start(cmask[:], cmask_i[:, :])
        for i in range(NT):
            nc.scalar.dma_start(dec_sb[i][:], dec_i[128 * i:128 * (i + 1), :])

        # ------------------------------------------------------- x0 loads
        for d in range(DC):
            nc.sync.dma_start(enc_sb[d][:], enc_i[128 * d:128 * (d + 1), :])
            nc.sync.dma_start(encv_sb[d][:],
                              encv_i[128 * d:128 * (d + 1), :])
            nc.sync.dma_start(lmh_sb[d][:], lmh_i[128 * d:128 * (d + 1), :])
        for tb in range(TB):
            nc.sync.dma_start(x_t16[tb][:], x0t_i[128 * tb:128 * (tb + 1), :])
        for th in range(2):
            for d in range(DC):
                nc.sync.dma_start(x_d16h[th][d][:],
                                  x0d_i[128 * d:128 * (d + 1),
                                        TH * th:TH * (th + 1)])

        # ===================================================== layer pieces
        def emit_phase_th(layer, th, acc, j_lo=0, j_hi=NJ):
            """Encoder+rope j-loop segment [j_lo, j_hi) for token half `th`,
            with depth-2 pipelined scores into `acc` (A' strips for th=0,
            B' strips for th=1).  Flushes trailing pairs when j_hi==NJ."""
            xs = xs_spill[layer % 2]
            lay = A_LAY if th == 0 else B_LAY

            def scores_pair(p):
                for kb in range(4 if th == 0 else 5):
                    tag, off = lay[kb]
                    if th == 0:
                        w = (4 - kb) * 128
                        q0 = 128 * kb
                    else:
                        w = 512
                        q0 = 512
                    nc.tensor.matmul(
                        acc[tag][:, off:off + w],
                        QR8[p][:, :, 128 * kb:128 * (kb + 1)],
                        QR8[p][:, :, q0:q0 + w],
                        start=(p == 0), stop=(p == NJ - 1),
                        perf_mode=DR)

            for j in range(j_lo, j_hi):
                cs2 = wp.tile([128, T], F16, name="cs2", tag="cs2", bufs=4)
                nc.sync.dma_start(cs2[:],
                                  cos2_i[128 * j:128 * (j + 1),
                                         T * th:T * (th + 1)])
                xs2 = wp.tile([128, T], F16, name="xs2", tag="xs2", bufs=5)
                for par in range(2):
                    nt = 2 * j + par
                    ps_e = psw(f"ps_enc_{layer}_{th}_{nt}")
                    for d in range(DC):
                        nc.tensor.matmul(
                            ps_e[:],
                            enc_sb[d][:, 128 * nt:128 * (nt + 1)],
                            x_d16h[th][d][:],
                            start=(d == 0), stop=(d == DC - 1))
                    nc.scalar.activation(xs2[:, TH * par:TH * (par + 1)],
                                         ps_e[:], ACTF.Relu)
                nc.sync.dma_start(
                    xs[256 * j:256 * (j + 1),
                       TH * th:TH * (th + 1)].rearrange(
                        "(b p) n -> p b n", p=128),
                    xs2[:].rearrange("p (b n) -> p b n", n=TH))
                # rope: cs2 = [c|s], xs2 = [xe|xo]; m2 split DVE/GpSimd to
                # balance engine load (j-loop is rope-throughput paced).
                # qe/qo land in fp16 (1-byte writes are slow on DVE/GpSimd);
                # one wide ACT copy converts both into the fp8 DR layout.
                m1 = wp.tile([128, T], F16, name="m1", tag="rope_m", bufs=6)
                nc.vector.tensor_mul(m1[:], xs2[:], cs2[:])
                m3 = wp.tile([128, T], F16, name="m3", tag="rope_q", bufs=6)
                nc.vector.tensor_sub(m3[:, 0:TH], m1[:, 0:TH], m1[:, TH:T])
                m2 = wp.tile([128, T], F16, name="m2", tag="rope_m", bufs=6)
                nc.gpsimd.tensor_mul(m2[:, 0:TH], xs2[:, TH:T], cs2[:, 0:TH])
                nc.vector.tensor_mul(m2[:, TH:T], xs2[:, 0:TH], cs2[:, TH:T])
                nc.gpsimd.tensor_add(m3[:, TH:T], m2[:, 0:TH], m2[:, TH:T])
                nc.scalar.activation(
                    QR8[j][:, :, TH * th:TH * (th + 1)],
                    m3[:].rearrange("p (s n) -> p s n", s=2),
                    ACTF.Copy)
                if j >= 4:
                    scores_pair(j - 4)
            if j_hi == NJ:
                for p in (NJ - 4, NJ - 3, NJ - 2, NJ - 1):
                    scores_pair(p)

        def spill_A(acc):
            for kb in range(4):
                tag, off = A_LAY[kb]
                w = (4 - kb) * 128
                nc.vector.tensor_copy(ST_lo[kb][:], acc[tag][:, off:off + w])
                nc.gpsimd.tensor_mul(ST_lo[kb][:, 0:128],
                                     ST_lo[kb][:, 0:128], cmask[:])

        def spill_B(acc):
            for kb in range(5):
                tag, off = B_LAY[kb]
                nc.vector.tensor_copy(ST_hi[kb][:], acc[tag][:, off:off + 512])
            nc.gpsimd.tensor_mul(ST_hi[4][:, 0:128],
                                 ST_hi[4][:, 0:128], cmask[:])

        def spill_C(acc):
            for kb in range(5, 8):
                tag, off = C_LAY[kb]
                w = (8 - kb) * 128
                nc.vector.tensor_copy(ST_hi[kb][:], acc[tag][:, off:off + w])
                nc.gpsimd.tensor_mul(ST_hi[kb][:, 0:128],
                                     ST_hi[kb][:, 0:128], cmask[:])

        def scores_C(acc):
            for p in range(NJ):
                for kb in range(5, 8):
                    tag, off = C_LAY[kb]
                    w = (8 - kb) * 128
                    nc.tensor.matmul(
                        acc[tag][:, off:off + w],
                        QR8[p][:, :, 128 * kb:128 * (kb + 1)],
                        QR8[p][:, :, 128 * kb:128 * kb + w],
                        start=(p == 0), stop=(p == NJ - 1),
                        perf_mode=DR)

        def ykv_half0(layer, yh):
            # ykvT partial, cols q in [0,512): strips kb 0..3 (ST_lo)
            for dc in range(DC):
                for kb in range(4):
                    w = (4 - kb) * 128
                    nc.tensor.matmul(
                        yh[:, 512 * dc + 128 * kb:512 * dc + 512],
                        x_t16[kb][:, 128 * dc:128 * (dc + 1)],
                        ST_lo[kb][:, 0:w],
                        start=(kb == 0), stop=(kb == 3))
            for dc in range(DC):
                yk = wp.tile([128, TH], F16, name="yk0", tag="yk_sb", bufs=2)
                nc.scalar.mul(yk[:], yh[:, 512 * dc:512 * dc + 512], ALPHA)
                nc.scalar.dma_start(ykv_ins[0][128 * dc:128 * (dc + 1), :],
                                    yk[:])
            nc.gpsimd.collective_compute(
                "AllReduce", ALU.add, replica_groups=pair_groups,
                ins=[ykv_ins[0].opt()], outs=[ykv_outs[layer][0].opt()])

        def ykv_half1(layer, yh):
            # ykvT partial, cols q in [512,1024): strips kb 0..7 (ST_hi)
            for dc in range(DC):
                for kb in range(8):
                    a = max(512, 128 * kb)
                    w = 1024 - a
                    nc.tensor.matmul(
                        yh[:, 512 * dc + a - 512:512 * dc + 512],
                        x_t16[kb][:, 128 * dc:128 * (dc + 1)],
                        ST_hi[kb][:, 0:w],
                        start=(kb == 0), stop=(kb == 7))
            for dc in range(DC):
                yk = wp.tile([128, TH], F16, name="yk1", tag="yk_sb", bufs=2)
                nc.scalar.mul(yk[:], yh[:, 512 * dc:512 * dc + 512], ALPHA)
                nc.scalar.dma_start(ykv_ins[1][128 * dc:128 * (dc + 1), :],
                                    yk[:])
            nc.gpsimd.collective_compute(
                "AllReduce", ALU.add, replica_groups=pair_groups,
                ins=[ykv_ins[1].opt()], outs=[ykv_outs[layer][1].opt()])

        def load_stats_half(layer, h):
            # load reduced ykvT half and compute per-token 1/std (deferred
            # inner LayerNorm; rows zero-mean so var = E[y^2])
            for dc in range(DC):
                nc.sync.dma_start(
                    ykvT[dc][:, TH * h:TH * (h + 1)],
                    ykv_outs[layer][h][128 * dc:128 * (dc + 1), :])
            ssq_ps = psW.tile([1, 512], F32, name=f"ssq_{layer}_{h}",
                              tag="ps_w", padded_shape=[128, 512], bufs=3)
            for dc in range(DC):
                sqt = wp.tile([128, TH], F16, name="sqt", tag="sqt", bufs=1)
                nc.scalar.activation(sqt[:], ykvT[dc][:, TH * h:TH * (h + 1)],
                                     ACTF.Square, scale=BETA)
                nc.tensor.matmul(ssq_ps[:], ones_t[:], sqt[:],
                                 start=(dc == 0), stop=(dc == DC - 1))
            std_row = wp.tile([1, TH], F32, name="std_row", tag="r_a",
                              bufs=2)
            nc.scalar.activation(std_row[:], ssq_ps[:], ACTF.Sqrt,
                                 bias=eps2_t[:],
                                 scale=1.0 / (D * BETA * BETA))
            inv_row = wp.tile([1, TH], F32, name="inv_row", tag="r_b",
                              bufs=2)
            nc.vector.reciprocal_approx_fast(inv_row[:], std_row[:])
            ib = wp.tile([128, TH], F32, name=f"inv_b{h}", tag=f"inv_b{h}",
                         bufs=1)
            nc.gpsimd.partition_broadcast(ib[:], inv_row[:])
            return ib

        def phase3_pass(layer, ci, ym_acc):
            lo, w, _ = YM_CHUNKS[ci]
            xs = xs_spill[layer % 2]
            prev = None
            xs_c2 = None

            def emit_ym(nt, xy):
                for dh in range(DC):
                    nc.tensor.matmul(
                        ym_acc[dh][:, lo:lo + w],
                        dec_sb[nt][:, 128 * dh:128 * (dh + 1)],
                        xy[:], start=(nt == 0), stop=(nt == NT - 1))

            for nt in range(NT):
                ps_v = psw(f"ps_ysp_{layer}_{nt}_{ci}", (128, w))
                for d in range(DC):
                    nc.tensor.matmul(
                        ps_v[:], encv_sb[d][:, 128 * nt:128 * (nt + 1)],
                        ykvT[d][:, lo:lo + w],
                        start=(d == 0), stop=(d == DC - 1))
                if nt % 2 == 0:
                    xs_c2 = wp.tile([128, 2 * w], F16, name="xs_c",
                                    tag="xs_c", bufs=3,
                                    padded_shape=[128, 1024])
                    nc.sync.dma_start(
                        xs_c2[:].rearrange("p (b n) -> p b n", n=w),
                        xs[128 * nt:128 * (nt + 2), lo:lo + w].rearrange(
                            "(b p) n -> p b n", p=128))
                xs_ap = xs_c2[:, w * (nt % 2):w * (nt % 2) + w]
                xy = wp.tile([128, w], F16, name="xy", tag="xy", bufs=4,
                             padded_shape=[128, 512])
                nc.vector.scalar_tensor_tensor(
                    xy[:], ps_v[:], 0.0, xs_ap, op0=ALU.max, op1=ALU.mult)
                if prev is not None:
                    emit_ym(*prev)
                prev = (nt, xy)
            emit_ym(*prev)

        def ym_reduce(layer, ci, ib, ym_acc):
            lo, w, _ = YM_CHUNKS[ci]
            for dh in range(DC):
                ym_sb = wp.tile([128, w], F16, name=f"ym_sb{ci}",
                                tag="ym_sb", bufs=2, padded_shape=[128, 512])
                nc.vector.tensor_mul(ym_sb[:], ym_acc[dh][:, lo:lo + w],
                                     ib[:, lo % 512:lo % 512 + w])
                nc.scalar.dma_start(ym_ins[ci][128 * dh:128 * (dh + 1), :],
                                    ym_sb[:])
            nc.gpsimd.collective_compute(
                "AllReduce", ALU.add, replica_groups=all_group,
                ins=[ym_ins[ci].opt()], outs=[ym_outs[layer][ci].opt()])

        def tail_first(layer, ci):
            """d-major tail, part 1: load u, column stats (PE-ones), first
            LN chain up to v = (u - mu)*inv1 + x_old.  No PE work after the
            mu/sq matmuls, so subsequent PE instructions (pass2 etc.) are
            not blocked behind the DVE latency chain."""
            lo, w, tbs = YM_CHUNKS[ci]
            th = lo // TH
            l0 = lo - TH * th
            ym_out = ym_outs[layer][ci]
            u = [wp.tile([128, w], F16, name=f"u{ci}_{dc}", tag=f"u_{dc}",
                         bufs=1, padded_shape=[128, 512]) for dc in range(DC)]
            for dc in range(DC):
                nc.sync.dma_start(u[dc][:],
                                  ym_out[128 * dc:128 * (dc + 1), :])
            # column stats over d (partition dim) via PE-ones matmuls
            mu_ps = psw(f"mu_ps_{layer}_{ci}", (1, w))
            for dc in range(DC):
                nc.tensor.matmul(mu_ps[:], ones_t[:], u[dc][:],
                                 start=(dc == 0), stop=(dc == DC - 1))
            sq_ps = psw(f"sq_ps_{layer}_{ci}", (1, w))
            for dc in range(DC):
                squ = wp.tile([128, w], F16, name="squ", tag="squ", bufs=2,
                              padded_shape=[128, 512])
                nc.scalar.activation(squ[:], u[dc][:], ACTF.Square)
                nc.tensor.matmul(sq_ps[:], ones_t[:], squ[:],
                                 start=(dc == 0), stop=(dc == DC - 1))
            mu_row = wp.tile([1, w], F32, name="mu_row", tag="r_mu", bufs=2)
            nc.scalar.mul(mu_row[:], mu_ps[:], 1.0 / D)
            mu_b = wp.tile([128, w], F32, name="mu_b", tag="mu_b", bufs=1,
                           padded_shape=[128, 512])
            nc.gpsimd.partition_broadcast(mu_b[:], mu_row[:])
            msq = wp.tile([1, w], F32, name="msq", tag="r_a", bufs=2)
            nc.vector.tensor_mul(msq[:], mu_row[:], mu_row[:])
            # vr2 = ssq/D - mu^2  (one fused DVE op, PSUM source)
            vr2 = wp.tile([1, w], F32, name="vr2", tag="r_b", bufs=2)
            nc.vector.scalar_tensor_tensor(vr2[:], sq_ps[:], 1.0 / D,
                                           msq[:], op0=ALU.mult,
                                           op1=ALU.subtract)
            std1 = wp.tile([1, w], F32, name="std1", tag="r_a", bufs=2)
            nc.scalar.activation(std1[:], vr2[:], ACTF.Sqrt,
                                 bias=eps_t[0:1, :])
            inv1 = wp.tile([1, w], F32, name="inv1", tag="r_b", bufs=2)
            nc.vector.reciprocal_approx_fast(inv1[:], std1[:])
            inv1_b = wp.tile([128, w], F32, name="inv1_b", tag="inv1_b",
                             bufs=1, padded_shape=[128, 512])
            nc.gpsimd.partition_broadcast(inv1_b[:], inv1[:])
            # v = (u - mu)*inv1 + x_old ;  second LN has exactly-zero mean
            v = [wp.tile([128, w], F16, name=f"v{ci}_{dc}", tag=f"v_{dc}",
                         bufs=1, padded_shape=[128, 512]) for dc in range(DC)]
            for dc in range(DC):
                d1 = wp.tile([128, w], F16, name="d1", tag="d1", bufs=1,
                             padded_shape=[128, 512])
                nc.vector.tensor_sub(d1[:], u[dc][:], mu_b[:])
                t2 = wp.tile([128, w], F16, name="t2", tag="t2", bufs=1,
                             padded_shape=[128, 512])
                nc.vector.tensor_mul(t2[:], d1[:], inv1_b[:])
                nc.gpsimd.tensor_add(v[dc][:], t2[:],
                                     x_d16h[th][dc][:, l0:l0 + w])
            return v

        def tail_second(layer, ci, v, last=False):
            """d-major tail, part 2: second LN (exactly-zero mean) and the
            x_d16h / x_t16 writes."""
            lo, w, tbs = YM_CHUNKS[ci]
            th = lo // TH
            l0 = lo - TH * th
            sq2_ps = psw(f"sq2_ps_{layer}_{ci}", (1, w))
            for dc in range(DC):
                sq2 = wp.tile([128, w], F16, name="sq2", tag="squ", bufs=2,
                              padded_shape=[128, 512])
                nc.scalar.activation(sq2[:], v[dc][:], ACTF.Square)
                nc.tensor.matmul(sq2_ps[:], ones_t[:], sq2[:],
                                 start=(dc == 0), stop=(dc == DC - 1))
            std2 = wp.tile([1, w], F32, name="std2", tag="r_mu", bufs=2)
            nc.scalar.activation(std2[:], sq2_ps[:], ACTF.Sqrt,
                                 bias=eps_t[0:1, :], scale=1.0 / D)
            inv2 = wp.tile([1, w], F32, name="inv2", tag="r_a", bufs=2)
            nc.vector.reciprocal_approx_fast(inv2[:], std2[:])
            inv2_b = wp.tile([128, w], F32, name="inv2_b", tag="inv2_b",
                             bufs=2, padded_shape=[128, 512])
            nc.gpsimd.partition_broadcast(inv2_b[:], inv2[:])
            for dc in range(DC):
                nc.vector.tensor_mul(x_d16h[th][dc][:, l0:l0 + w], v[dc][:],
                                     inv2_b[:])
            # t-major x for the ykv matmuls (off critical path)
            for tb in tbs:
                for dc in range(DC):
                    nc.sync.dma_start_transpose(
                        x_t16[tb][:, 128 * dc:128 * (dc + 1)],
                        x_d16h[th][dc][:, 128 * (tb % 4):
                                       128 * (tb % 4) + 128])
            # lm head for this chunk's token blocks (final layer only)
            if last:
                for tb in tbs:
                    ps_l = psw(f"ps_lg_{tb}", (128, VOCAB))
                    for d in range(DC):
                        nc.tensor.matmul(
                            ps_l[:],
                            x_d16h[th][d][:, 128 * (tb % 4):
                                          128 * (tb % 4) + 128],
                            lmh_sb[d][:], start=(d == 0), stop=(d == DC - 1))
                    lg_sb = wp.tile([128, VOCAB], F32, name="lg_sb",
                                    tag="lg_sb")
                    nc.vector.tensor_copy(lg_sb[:], ps_l[:])
                    nc.sync.dma_start(out_o[128 * tb:128 * (tb + 1), :],
                                      lg_sb[:])

        # ============================================================ layers
        def phaseA_finish(layer, acc):
            spill_A(acc)
            yh0 = sacc("S3", f"S3_y0_{layer}")
            ykv_half0(layer, yh0)

        def emit_rest(layer):
            last = layer == n_layer - 1
            accB = {t: sacc(t, f"{t}_B_{layer}") for t in ("S1", "S2", "S3")}
            emit_phase_th(layer, 1, accB)
            spill_B(accB)
            accC = {"S3": sacc("S3", f"S3_C_{layer}")}
            scores_C(accC)
            spill_C(accC)
            yh1 = sacc("S1", f"S1_y1_{layer}")
            ykv_half1(layer, yh1)

            ib0 = load_stats_half(layer, 0)
            ym_acc = {0: sacc("S3", f"S3_ym_{layer}"),
                      1: sacc("S1", f"S1_ym_{layer}")}
            phase3_pass(layer, 0, ym_acc)
            ym_reduce(layer, 0, ib0, ym_acc)
            ib1 = load_stats_half(layer, 1)
            phase3_pass(layer, 1, ym_acc)
            ym_reduce(layer, 1, ib1, ym_acc)
            v0 = tail_first(layer, 0)
            phase3_pass(layer, 2, ym_acc)
            ym_reduce(layer, 2, ib1, ym_acc)
            tail_second(layer, 0, v0, last=last)
            v1 = tail_first(layer, 1)
            if layer + 1 < n_layer:
                # interleave next layer's PHASE A with the remaining tails
                accA = {t: sacc(t, f"{t}_A_{layer + 1}")
                        for t in ("S1", "S2")}
                emit_phase_th(layer + 1, 0, accA, 0, 8)
                tail_second(layer, 1, v1, last=last)
                emit_phase_th(layer + 1, 0, accA, 8, 12)
                v2 = tail_first(layer, 2)
                tail_second(layer, 2, v2, last=last)
                emit_phase_th(layer + 1, 0, accA, 12, NJ)
                phaseA_finish(layer + 1, accA)
            else:
                tail_second(layer, 1, v1, last=last)
                v2 = tail_first(layer, 2)
                tail_second(layer, 2, v2, last=last)

        accA0 = {t: sacc(t, f"{t}_A_0") for t in ("S1", "S2")}
        emit_phase_th(0, 0, accA0)
        phaseA_finish(0, accA0)
        for layer in range(n_layer):
            emit_rest(layer)

    nc.compile()
    return nc


# ------------------------------------------------------------- host helpers
def _host_tables():
    """cos/sin rope tables in [pair, t] layout, mirroring reference fp32 math."""
    n = np.arange(N, dtype=np.float32)
    q = np.floor(n / 2.0) * 2.0
    freqs = (1.0 / (np.float32(THETA) ** (q / np.float32(N)))
             / np.float32(2.0 * math.pi)).astype(np.float32)
    t = np.arange(T, dtype=np.float32)
    phases = (t[:, None] * freqs[None, :]) % 1.0
    phases = phases * np.float32(2.0 * math.pi)
    cos = np.cos(phases).astype(np.float32)   # [T, N]
    sin = np.sin(phases).astype(np.float32)
    # pair p uses freq of n=2p; table[p, t]
    cos_p = cos[:, 0::2].T.copy()  # [N//2, T]
    sin_p = sin[:, 0::2].T.copy()
    return cos_p, sin_p


def _perm_local():
    """Local latent permutation: position -> (pair index, odd flag)."""
    pos_to_pair = np.empty(NHALF, dtype=np.int64)
    pos_is_odd = np.empty(NHALF, dtype=np.int64)
    for j in range(NJ):
        pr = np.arange(128) + 128 * j
        pos_to_pair[256 * j:256 * j + 128] = pr
        pos_is_odd[256 * j:256 * j + 128] = 0
        pos_to_pair[256 * j + 128:256 * j + 256] = pr
        pos_is_odd[256 * j + 128:256 * j + 256] = 1
    return pos_to_pair, pos_is_odd


_NC_CACHE = {}


def _get_nc():
    if "nc" not in _NC_CACHE:
        _NC_CACHE["nc"] = build_program()
    return _NC_CACHE["nc"]


def prepare_in_maps(idx, embed, encoder, encoder_v, decoder, lm_head):
    idx = np.asarray(idx)
    embed = np.asarray(embed, dtype=np.float32)
    encoder = np.asarray(encoder, dtype=np.float32)
    encoder_v = np.asarray(encoder_v, dtype=np.float32)
    decoder = np.asarray(decoder, dtype=np.float32)
    lm_head = np.asarray(lm_head, dtype=np.float32)

    cos_p, sin_p = _host_tables()
    pos_to_pair, pos_is_odd = _perm_local()

    cmask = (np.arange(128)[:, None] < np.arange(128)[None, :]).astype(np.float16)
    lmh16 = lm_head.astype(np.float16)

    # x0 = LN(embed)[idx] (host-side input prep, fp32 math as in reference)
    mu = embed.mean(axis=-1, keepdims=True)
    var = embed.var(axis=-1, keepdims=True)
    emb_n = (embed - mu) / np.sqrt(var + np.float32(EPS))
    x0 = emb_n[np.asarray(idx).reshape(T)]          # [T, D] f32
    x0_t = x0.astype(np.float16)
    x0_d = x0.T.copy().astype(np.float16)

    in_maps = []
    for c in range(NCORES):
        h, eta = c // 2, c % 2
        pair_g = NPAIR * eta + pos_to_pair          # global pair index
        n_orig = 2 * pair_g + pos_is_odd            # original n within head
        enc_sh = encoder[h][:, n_orig].astype(np.float16)
        encv_sh = encoder_v[h][:, n_orig].astype(np.float16)
        dec_sh = decoder[h * N + n_orig, :].astype(np.float16)
        cos_sh = cos_p[NPAIR * eta:NPAIR * (eta + 1), :].astype(np.float16)
        sin_sh = sin_p[NPAIR * eta:NPAIR * (eta + 1), :].astype(np.float16)
        # [c_th0 | s_th0 | c_th1 | s_th1]
        cos2 = np.concatenate([cos_sh[:, :TH], sin_sh[:, :TH],
                               cos_sh[:, TH:], sin_sh[:, TH:]], axis=1)
        in_maps.append({
            "x0_t": x0_t, "x0_d": x0_d, "enc_sh": enc_sh,
            "encv_sh": encv_sh, "dec_sh": dec_sh, "lmh": lmh16,
            "cos2_sh": cos2, "cmask": cmask,
        })
    return in_maps


def kernel(idx, embed, encoder, encoder_v, decoder, lm_head):
    in_maps = prepare_in_maps(idx, embed, encoder, encoder_v, decoder,
                              lm_head)
    nc = _get_nc()
    res = bass_utils.run_bass_kernel_spmd(nc, in_maps,
                                          core_ids=list(range(NCORES)))
    _NC_CACHE["last_results"] = res
    logits = np.asarray(res.results[0]["logits"], dtype=np.float32)
    return logits.reshape(1, T, VOCAB)
